# revision 1
# baseline (speedup 1.0000x reference)
"""MeshConv (gnn_message_passing) Bass kernel for 8 trn2 NeuronCores.

out[b,o,v] = bias[o] + sum_k coeffs[k,:,o]^T feats_k[b,v,:]
  feats_0 = x^T (identity), feats_{1,2,3} = spmm(L/EW/NS, x)

Strategy: shard output vertices across cores (row-partitioned spmm).
Edges sorted by destination row into 128-edge chunks per 128-row tile.
Per chunk: dma_gather of x rows (1KB rows, int16 indices split lo/hi
around row 32768, 4 SWDGE queues in parallel), a one-hot
[edge, row_local]*val matrix built on DVE with one fused tensor_scalar,
and a PE matmul accumulating y_k[row, (b,c)] in PSUM.  y is transposed on
PE and hit with the per-operator coeffs (free dim 256 => full-rate f32r),
bias added on DVE, output written as [o, rows] slabs per batch.
"""

import sys

sys.path.insert(0, "/opt/trn_rl_repo")

import numpy as np

import concourse.bass as bass
import concourse.bacc as bacc
import concourse.tile as tile
import concourse.mybir as mybir
from concourse.bass_utils import run_bass_kernel_spmd
from concourse.masks import make_identity

NV = 40962
B = 4
C = 64
BC = B * C  # 256
NCORES = 8
NTILE = 328          # 128-row tiles, 328*128 = 41984 >= 40962
NVPAD = NTILE * 128
TPC = NTILE // NCORES  # 41 tiles per core
SPLIT = 32768        # int16 index split point
MAXCH = 8            # dma_gather limit: <=1024 indices per call
NQ = 4               # SWDGE queues

MM_MODE = "f32r"     # "f32r" (fast, ~3e-4 rel err) or "f32" (exact, slower)

_cache = {}


def _trunc_f32r(a):
    return (a.view(np.uint32) & np.uint32(0xFFFFF000)).view(np.float32)


def _prep_op(row, col, val):
    """Sort edges by row; per (tile, half) bucket into 128-slot chunks.

    Slot layout per tile: [C_lo chunks | C_hi chunks]; slot (j, p) holds the
    (j*128+p)-th edge of its half-group.  Returns per-[NTILE, 128, C] arrays
    idx (int16, relative to half base), rloc (f32), val (f32) plus (C_lo,
    C_hi).
    """
    row = np.asarray(row).astype(np.int64)
    col = np.asarray(col).astype(np.int64)
    val = np.asarray(val).astype(np.float32)
    order = np.argsort(row, kind="stable")
    row, col, val = row[order], col[order], val[order]
    tile_id = row >> 7
    ishi = (col >= SPLIT).astype(np.int64)

    halves = []
    for h in (0, 1):
        m = ishi == h
        r_h, c_h, v_h, t_h = row[m], col[m], val[m], tile_id[m]
        counts = np.bincount(t_h, minlength=NTILE)
        Ch = int(np.ceil(max(int(counts.max()), 1) / 128))
        slots = Ch * 128
        starts = np.zeros(NTILE, np.int64)
        starts[1:] = np.cumsum(counts)[:-1]
        pos = np.arange(len(r_h)) - starts[t_h]
        flat = t_h * slots + pos
        idxP = np.zeros(NTILE * slots, np.int16)
        rlocP = np.zeros(NTILE * slots, np.float32)
        valP = np.zeros(NTILE * slots, np.float32)
        idxP[flat] = (c_h - h * SPLIT).astype(np.int16)
        rlocP[flat] = (r_h & 127).astype(np.float32)
        valP[flat] = v_h
        # [NTILE, C, 128] -> [NTILE, 128, C]
        halves.append((
            Ch,
            idxP.reshape(NTILE, Ch, 128).transpose(0, 2, 1),
            rlocP.reshape(NTILE, Ch, 128).transpose(0, 2, 1),
            valP.reshape(NTILE, Ch, 128).transpose(0, 2, 1),
        ))
    (C_lo, i_lo, r_lo, v_lo), (C_hi, i_hi, r_hi, v_hi) = halves
    idxP = np.concatenate([i_lo, i_hi], axis=2)
    rlocP = np.concatenate([r_lo, r_hi], axis=2)
    valP = np.concatenate([v_lo, v_hi], axis=2)
    return (C_lo, C_hi), idxP, rlocP, valP


def _wrap16(arr):
    """[n] int16 (n%16==0) -> [128, n//16]: wrapped in 16 partitions,
    replicated for the 8 gpsimd cores."""
    n = arr.shape[-1]
    t16 = arr.reshape(-1, n // 16, 16)
    t16 = np.swapaxes(t16, -1, -2)  # [..., 16, n//16]
    return np.tile(t16, (1, 8, 1)) if arr.ndim > 1 else np.tile(t16[0], (8, 1))


def _calls(S_ops):
    """Static per-tile gather call list: (op_i, chunk_off, nchunks, is_hi)."""
    calls = []
    off = 0
    for opi, (C_lo, C_hi) in enumerate(S_ops):
        for h, Ch in ((0, C_lo), (1, C_hi)):
            a = 0
            while a < Ch:
                n = min(MAXCH, Ch - a)
                calls.append((opi, off + a, n, h))
                a += n
            off += Ch
    return calls


def _build(S_ops):
    """Build the per-core Bass program for ((C_L_lo,C_L_hi),(..E..),(..N..))."""
    STOT = sum(c for p in S_ops for c in p)
    f32 = mybir.dt.float32
    f32r = mybir.dt.float32r if MM_MODE == "f32r" else mybir.dt.float32

    nc = bacc.Bacc("TRN2", target_bir_lowering=False, debug=False,
                   num_devices=NCORES, num_swdge_queues=NQ)

    xg_d = nc.dram_tensor("xg", [NVPAD, BC], f32r, kind="ExternalInput")
    xTown_d = nc.dram_tensor("xTown", [TPC * 128, BC], f32,
                             kind="ExternalInput")
    idx_d = nc.dram_tensor("idx16", [128, TPC * STOT * 8], mybir.dt.int16,
                           kind="ExternalInput")
    rloc_d = nc.dram_tensor("rloc", [128, TPC * STOT], f32,
                            kind="ExternalInput")
    val_d = nc.dram_tensor("val", [128, TPC * STOT], f32,
                           kind="ExternalInput")
    iota_d = nc.dram_tensor("iota", [128, 128], f32, kind="ExternalInput")
    coef_d = nc.dram_tensor("coef", [64, 256], f32, kind="ExternalInput")
    bias_d = nc.dram_tensor("bias2", [128, 1], f32, kind="ExternalInput")
    out_d = nc.dram_tensor("out", [B, C, TPC * 128], f32,
                           kind="ExternalOutput")

    calls = _calls(S_ops)
    OPNAMES = ["L", "E", "N"]
    # chunk index ranges per op
    op_off = []
    o = 0
    for C_lo, C_hi in S_ops:
        op_off.append((o, C_lo + C_hi))
        o += C_lo + C_hi

    with tile.TileContext(nc) as tc:
        with (
            tc.tile_pool(name="const", bufs=1) as cpool,
            tc.tile_pool(name="meta", bufs=1) as mpool,
            tc.tile_pool(name="g", bufs=2) as gpool,
            tc.tile_pool(name="oh", bufs=6) as ohpool,
            tc.tile_pool(name="ys", bufs=2) as yspool,
            tc.tile_pool(name="yt", bufs=2) as ytpool,
            tc.tile_pool(name="os", bufs=2) as ospool,
            tc.tile_pool(name="py", bufs=1, space="PSUM") as pypool,
            tc.tile_pool(name="pt", bufs=2, space="PSUM") as ptpool,
            tc.tile_pool(name="po", bufs=2, space="PSUM") as popool,
        ):
            # constants
            iota_t = cpool.tile([128, 128], f32)
            nc.sync.dma_start(iota_t[:], iota_d.ap()[:])
            ident_t = cpool.tile([128, 128], f32)
            make_identity(nc, ident_t[:])
            coef_f32 = cpool.tile([64, 256], f32)
            nc.sync.dma_start(coef_f32[:], coef_d.ap()[:])
            coef_t = cpool.tile([64, 256], f32r)
            nc.vector.tensor_copy(coef_t[:], coef_f32[:])
            bias_t = cpool.tile([128, 1], f32)
            nc.sync.dma_start(bias_t[:], bias_d.ap()[:])
            idx_t = mpool.tile([128, TPC * STOT * 8], mybir.dt.int16)
            nc.sync.dma_start(idx_t[:], idx_d.ap()[:])
            rloc_t = mpool.tile([128, TPC * STOT], f32)
            nc.sync.dma_start(rloc_t[:], rloc_d.ap()[:])
            val_t = mpool.tile([128, TPC * STOT], f32)
            nc.sync.dma_start(val_t[:], val_d.ap()[:])

            yT = {}  # (k, b) -> staging tile [64, 256] across a tile pair
            qn = 0

            for t in range(TPC):
                mbase = t * STOT
                pair_off = (t % 2) * 128
                is_pair_start = t % 2 == 0
                is_orphan = t == TPC - 1 and is_pair_start

                g_t = gpool.tile([128, STOT * BC], f32r, tag="g")
                for opi, coff, nch, h in calls:
                    src = xg_d.ap()[SPLIT:, :] if h else xg_d.ap()[:SPLIT, :]
                    ib = (mbase + coff) * 8
                    nc.gpsimd.dma_gather(
                        out_ap=g_t[:, coff * BC:(coff + nch) * BC]
                        .rearrange("p (j f) -> p j f", f=BC),
                        in_ap=src,
                        idxs_ap=idx_t[:, ib:ib + nch * 8],
                        num_idxs=nch * 128,
                        num_idxs_reg=nch * 128,
                        elem_size=BC,
                        queue_num=qn % NQ,
                    )
                    qn += 1

                # identity features: dense rows of this core's xT slice
                ident_rows = yspool.tile([128, BC], f32, tag="yI")
                nc.sync.dma_start(
                    ident_rows[:], xTown_d.ap()[t * 128:(t + 1) * 128, :])

                # chunk matmuls per op
                y_sb = {"I": ident_rows}
                for opi, op in enumerate(OPNAMES):
                    coff, S_op = op_off[opi]
                    py_t = pypool.tile([128, BC], f32, tag=f"y{op}")
                    for j in range(S_op):
                        oh_t = ohpool.tile([128, 128], f32r, tag="oh")
                        mcol = mbase + coff + j
                        nc.vector.tensor_scalar(
                            out=oh_t[:],
                            in0=iota_t[:],
                            scalar1=rloc_t[:, mcol:mcol + 1],
                            scalar2=val_t[:, mcol:mcol + 1],
                            op0=mybir.AluOpType.is_equal,
                            op1=mybir.AluOpType.mult,
                        )
                        nc.tensor.matmul(
                            py_t[:],
                            oh_t[:],
                            g_t[:, (coff + j) * BC:(coff + j + 1) * BC],
                            start=(j == 0),
                            stop=(j == S_op - 1),
                        )
                    ys_t = yspool.tile([128, BC], f32, tag=f"ys{op}")
                    nc.scalar.activation(ys_t[:], py_t[:],
                                         mybir.ActivationFunctionType.Copy)
                    y_sb[op] = ys_t

                # transpose y[128r, 256bc] -> yT[(k,b)][64c, 128r]
                for ki, k in enumerate(["I", "L", "E", "N"]):
                    for b in range(B):
                        if is_pair_start:
                            yT[(k, b)] = ytpool.tile(
                                [64, 256], f32r, tag=f"yT{k}{b}",
                                name=f"yT{k}{b}_{t}")
                            if is_orphan:
                                nc.vector.memset(
                                    yT[(k, b)][:].bitcast(mybir.dt.float32),
                                    0.0)
                        pt_t = ptpool.tile([64, 128], f32, tag="psT")
                        nc.tensor.transpose(
                            pt_t[:], y_sb[k][:, b * 64:(b + 1) * 64],
                            ident_t[:])
                        nc.scalar.activation(
                            yT[(k, b)][:, pair_off:pair_off + 128], pt_t[:],
                            mybir.ActivationFunctionType.Copy)

                # coeffs matmuls on completed pair
                if not is_pair_start or is_orphan:
                    r0 = (t - 1 if not is_pair_start else t) * 128
                    ncols = 128 if is_orphan else 256
                    for b in range(B):
                        po_t = popool.tile([64, 256], f32, tag="po",
                                           name=f"po{b}_{t}")
                        for ki, k in enumerate(["I", "L", "E", "N"]):
                            nc.tensor.matmul(
                                po_t[:],
                                coef_t[:, ki * 64:(ki + 1) * 64],
                                yT[(k, b)][:],
                                start=(ki == 0),
                                stop=(ki == 3),
                            )
                        os_t = ospool.tile([64, 256], f32, tag="os",
                                           name=f"os{b}_{t}")
                        nc.vector.tensor_scalar(
                            out=os_t[:], in0=po_t[:],
                            scalar1=bias_t[0:64, :1], scalar2=None,
                            op0=mybir.AluOpType.add)
                        nc.sync.dma_start(
                            out_d.ap()[b:b + 1, :, r0:r0 + ncols]
                            .rearrange("b o r -> (b o) r"),
                            os_t[:, :ncols])

    nc.compile()
    return nc


def kernel(**inputs):
    x = np.asarray(inputs["x"], dtype=np.float32)
    coeffs = np.asarray(inputs["coeffs"], dtype=np.float32)
    bias = np.asarray(inputs["bias"], dtype=np.float32)

    xT = np.zeros((NVPAD, BC), np.float32)
    xT[:NV] = x.transpose(2, 0, 1).reshape(NV, BC)
    xg = _trunc_f32r(xT) if MM_MODE == "f32r" else xT

    ops = []
    for name in ("L", "EW", "NS"):
        S, idxP, rlocP, valP = _prep_op(
            inputs[f"{name}_row"], inputs[f"{name}_col"], inputs[f"{name}_val"])
        ops.append((S, idxP, rlocP, valP))
    S_ops = tuple(o[0] for o in ops)

    key = (S_ops, MM_MODE)
    if key not in _cache:
        _cache[key] = _build(S_ops)
    nc = _cache[key]

    iota = np.broadcast_to(np.arange(128, dtype=np.float32), (128, 128)).copy()
    coef_in = coeffs.transpose(1, 0, 2).reshape(64, 256).copy()  # [c, k*64+o]
    bias2 = np.tile(bias, 2).reshape(128, 1).astype(np.float32)

    in_maps = []
    for core in range(NCORES):
        t0, t1 = core * TPC, (core + 1) * TPC
        # idx16: per tile, per op: [128, C*8] wrapped-16 layout
        idx_parts = []
        for t in range(t0, t1):
            for o in ops:
                arr = o[1][t]  # [128, C] slot layout [p, j]: edge j*128+p
                flat = arr.transpose(1, 0).reshape(-1)  # [C*128] edge order
                idx_parts.append(_wrap16(flat))
        idx16 = np.concatenate(idx_parts, axis=1)
        rloc = np.concatenate(
            [np.concatenate([o[2][t] for o in ops], axis=1)
             for t in range(t0, t1)], axis=1)
        val = np.concatenate(
            [np.concatenate([o[3][t] for o in ops], axis=1)
             for t in range(t0, t1)], axis=1)
        in_maps.append({
            "xg": xg,
            "xTown": np.ascontiguousarray(xT[t0 * 128:t1 * 128]),
            "idx16": np.ascontiguousarray(idx16),
            "rloc": np.ascontiguousarray(rloc),
            "val": np.ascontiguousarray(val),
            "iota": iota, "coef": coef_in, "bias2": bias2,
        })

    res = run_bass_kernel_spmd(nc, in_maps, core_ids=list(range(NCORES)))
    out = np.concatenate([res.results[c]["out"] for c in range(NCORES)],
                         axis=2)
    return np.ascontiguousarray(out[:, :, :NV])



# revision 5
# speedup vs baseline: 1.3034x; 1.3034x over previous
"""MeshConv (gnn_message_passing) Bass kernel for 8 trn2 NeuronCores.

out[b,o,v] = bias[o] + sum_k coeffs[k,:,o]^T feats_k[b,v,:]
  feats_0 = x^T (identity), feats_{1,2,3} = spmm(L/EW/NS, x)

Strategy: shard output vertices across cores (row-partitioned spmm),
fp16 data path.  Edges sorted by (dest tile, col-half, col) into 128-edge
chunks; per tile one dma_gather per <=8-chunk range pulls 512B fp16 rows
of x^T (int16 indices split lo/hi around row 32768).  One-hot matrices
[edge, row_local]*val are precomputed on the host in fp16 and streamed
from HBM (no DVE work).  Per chunk the PE computes
yT[bc_half, row] += g[edge, bc_half]^T @ oh[edge, row] directly in the
transposed layout, so no PE transposes are needed; identity features are
DMA'd straight from x (natural [bc, v] layout).  Per tile pair the
per-operator coeffs hit yT (free dim 256), bias added on the Scalar
engine, output written as [o, rows] slabs per batch.
"""

import sys

sys.path.insert(0, "/opt/trn_rl_repo")

import numpy as np

import concourse.bass as bass
import concourse.bacc as bacc
import concourse.tile as tile
import concourse.mybir as mybir
from concourse.bass_utils import run_bass_kernel_spmd

NV = 40962
B = 4
C = 64
BC = B * C  # 256
NCORES = 8
NTILE = 328          # 128-row tiles, 328*128 = 41984 >= 40962
NVPAD = NTILE * 128
TPC = NTILE // NCORES  # 41 tiles per core
TPCR = TPC * 128
SPLIT = 32768        # int16 index split point
MAXD = 8             # max chunks per dma_gather call (8*128 = 1024 descs)
NQ = 4               # SWDGE queues (ucode max)

OH_MODE = "stream"   # "stream": fp16 one-hots from HBM; "dve": build on DVE

_cache = {}

OPK = ("L", "E", "N")


def _calls(CLO, STOT):
    """Static per-tile gather call list: (chunk_off, nchunks, is_hi)."""
    calls = []
    for lo, hi in ((0, CLO), (CLO, STOT)):
        a = lo
        while a < hi:
            n = min(MAXD, hi - a)
            calls.append((a, n, lo != 0))
            a += n
    return calls


def _build(S, oh_mode):
    """Build the per-core Bass program for ((CL0,CL1),(CE0,CE1),(CN0,CN1))."""
    C0s = [c0 for c0, _ in S]
    C1s = [c1 for _, c1 in S]
    CLO, CHI = sum(C0s), sum(C1s)
    STOT = CLO + CHI
    lo_base = [0, C0s[0], C0s[0] + C0s[1]]
    hi_base = [CLO, CLO + C1s[0], CLO + C1s[0] + C1s[1]]
    op_chunks = {
        k: list(range(lo_base[i], lo_base[i] + C0s[i]))
        + list(range(hi_base[i], hi_base[i] + C1s[i]))
        for i, k in enumerate(OPK)
    }
    f32 = mybir.dt.float32
    f16 = mybir.dt.float16

    nc = bacc.Bacc("TRN2", target_bir_lowering=False, debug=False,
                   num_devices=NCORES, num_swdge_queues=NQ)

    xg_d = nc.dram_tensor("xg", [NVPAD, BC], f16, kind="ExternalInput")
    xI_d = nc.dram_tensor("xI", [128, 2 * TPCR], f16, kind="ExternalInput")
    idx_d = nc.dram_tensor("idx16", [128, TPC * STOT * 8], mybir.dt.int16,
                           kind="ExternalInput")
    if oh_mode == "stream":
        oh_d = nc.dram_tensor("oh", [128, TPC * STOT * 128], f16,
                              kind="ExternalInput")
    else:
        iota_d = nc.dram_tensor("iota16", [128, 128], f16,
                                kind="ExternalInput")
        rloc_d = nc.dram_tensor("rloc", [128, TPC * STOT], f16,
                                kind="ExternalInput")
        val_d = nc.dram_tensor("val", [128, TPC * STOT], f16,
                               kind="ExternalInput")
    coef_d = nc.dram_tensor("coef", [128, 256], f16, kind="ExternalInput")
    bias_d = nc.dram_tensor("bias2", [64, 1], f32, kind="ExternalInput")
    out_d = nc.dram_tensor("out", [B, C, TPC * 128], f32,
                           kind="ExternalOutput")

    calls = _calls(CLO, STOT)

    with tile.TileContext(nc) as tc:
        with (
            tc.tile_pool(name="const", bufs=1) as cpool,
            tc.tile_pool(name="meta", bufs=1) as mpool,
            tc.tile_pool(name="g", bufs=2) as gpool,
            tc.tile_pool(name="oh", bufs=2) as ohpool,
            tc.tile_pool(name="yt", bufs=2) as ytpool,
            tc.tile_pool(name="os", bufs=2) as ospool,
            tc.tile_pool(name="py", bufs=1, space="PSUM") as pypool,
            tc.tile_pool(name="po", bufs=2, space="PSUM") as popool,
        ):
            # constants
            coef_t = cpool.tile([128, 256], f16)
            nc.sync.dma_start(coef_t[:], coef_d.ap()[:])
            bias_t = cpool.tile([64, 1], f32)
            nc.sync.dma_start(bias_t[:], bias_d.ap()[:])
            idx_t = mpool.tile([128, TPC * STOT * 8], mybir.dt.int16)
            nc.sync.dma_start(idx_t[:], idx_d.ap()[:])
            if oh_mode != "stream":
                iota_t = cpool.tile([128, 128], f16)
                nc.sync.dma_start(iota_t[:], iota_d.ap()[:])
                rloc_t = mpool.tile([128, TPC * STOT], f16)
                nc.sync.dma_start(rloc_t[:], rloc_d.ap()[:])
                val_t = mpool.tile([128, TPC * STOT], f16)
                nc.sync.dma_start(val_t[:], val_d.ap()[:])

            yts = {}  # (k, hb) -> staging tile [128, 256] f16 per tile pair
            qn = 0

            for t in range(TPC):
                pair_off = (t % 2) * 128
                is_pair_start = t % 2 == 0
                is_orphan = t == TPC - 1 and is_pair_start

                g_t = gpool.tile([128, STOT * BC], f16, tag="g")
                ib = t * STOT * 8
                for coff, nch, hi in calls:
                    src = xg_d.ap()[SPLIT:, :] if hi else xg_d.ap()[:SPLIT, :]
                    nc.gpsimd.dma_gather(
                        out_ap=g_t[:, coff * BC:(coff + nch) * BC]
                        .rearrange("p (j f) -> p j f", f=BC),
                        in_ap=src,
                        idxs_ap=idx_t[:, ib + coff * 8:ib + (coff + nch) * 8],
                        num_idxs=nch * 128,
                        num_idxs_reg=nch * 128,
                        elem_size=BC,
                        queue_num=qn % NQ,
                    )
                    qn += 1

                oh_t = ohpool.tile([128, STOT * 128], f16, tag="oh")
                if oh_mode == "stream":
                    nc.sync.dma_start(
                        oh_t[:],
                        oh_d.ap()[:, t * STOT * 128:(t + 1) * STOT * 128])
                else:
                    for cj in range(STOT):
                        mcol = t * STOT + cj
                        nc.vector.tensor_scalar(
                            out=oh_t[:, cj * 128:(cj + 1) * 128],
                            in0=iota_t[:],
                            scalar1=rloc_t[:, mcol:mcol + 1],
                            scalar2=val_t[:, mcol:mcol + 1],
                            op0=mybir.AluOpType.is_equal,
                            op1=mybir.AluOpType.mult,
                        )

                if is_pair_start:
                    for k in ("I",) + OPK:
                        for hb in (0, 1):
                            yts[(k, hb)] = ytpool.tile(
                                [128, 256], f16, tag=f"yt{k}{hb}",
                                name=f"yt{k}{hb}_{t}")
                            if is_orphan:
                                nc.vector.memset(
                                    yts[(k, hb)][:].bitcast(f32), 0.0)

                # identity features straight from x (natural layout)
                for hb in (0, 1):
                    nc.sync.dma_start(
                        yts[("I", hb)][:, pair_off:pair_off + 128],
                        xI_d.ap()[:, hb * TPCR + t * 128:
                                  hb * TPCR + (t + 1) * 128])

                # spmm: yT[bc_half, row] += g[e, bc_half]^T @ oh[e, row]
                for k in OPK:
                    chunks = op_chunks[k]
                    for hb in (0, 1):
                        py_t = pypool.tile([128, 128], f32, tag=f"py{k}{hb}")
                        for ci, cj in enumerate(chunks):
                            nc.tensor.matmul(
                                py_t[:],
                                g_t[:, cj * BC + hb * 128:
                                    cj * BC + hb * 128 + 128],
                                oh_t[:, cj * 128:(cj + 1) * 128],
                                start=(ci == 0),
                                stop=(ci == len(chunks) - 1),
                            )
                        nc.scalar.activation(
                            yts[(k, hb)][:, pair_off:pair_off + 128], py_t[:],
                            mybir.ActivationFunctionType.Copy)

                # coeffs matmuls on completed pair
                if not is_pair_start or is_orphan:
                    r0 = (t - 1 if not is_pair_start else t) * 128
                    ncols = 128 if is_orphan else 256
                    for b in range(B):
                        po_t = popool.tile([64, 256], f32, tag="po",
                                           name=f"po{b}_{t}")
                        for ki, k in enumerate(("I",) + OPK):
                            p0 = (b % 2) * 64
                            rhs = yts[(k, b // 2)][p0:p0 + 64, :]
                            nc.tensor.matmul(
                                po_t[:],
                                coef_t[p0:p0 + 64, ki * 64:(ki + 1) * 64],
                                rhs,
                                start=(ki == 0),
                                stop=(ki == 3),
                            )
                        os_t = ospool.tile([64, 256], f32, tag="os",
                                           name=f"os{b}_{t}")
                        nc.scalar.activation(
                            os_t[:], po_t[:],
                            mybir.ActivationFunctionType.Identity,
                            bias=bias_t[:, :1])
                        nc.sync.dma_start(
                            out_d.ap()[b:b + 1, :, r0:r0 + ncols]
                            .rearrange("b o r -> (b o) r"),
                            os_t[:, :ncols])

    nc.compile()
    return nc


def _prep(inputs):
    """Sort edges by (tile, col-half, col); compute global chunk counts and
    fill flat slot arrays (idx, one-hot / rloc+val)."""
    ops = []
    for name in ("L", "EW", "NS"):
        row = np.asarray(inputs[f"{name}_row"]).astype(np.int64)
        col = np.asarray(inputs[f"{name}_col"]).astype(np.int64)
        val = np.asarray(inputs[f"{name}_val"]).astype(np.float32)
        t = row >> 7
        h = (col >= SPLIT).astype(np.int64)
        order = np.lexsort((col, h, t))
        row, col, val, t, h = (a[order] for a in (row, col, val, t, h))
        grp = t * 2 + h
        counts = np.bincount(grp, minlength=NTILE * 2)
        starts = np.zeros(NTILE * 2, np.int64)
        starts[1:] = np.cumsum(counts)[:-1]
        pos = np.arange(len(row)) - starts[grp]
        C0 = int(np.ceil(max(int(counts[0::2].max()), 1) / 128))
        C1 = int(np.ceil(max(int(counts[1::2].max()), 1) / 128))
        ops.append((row, col, val, t, h, pos, C0, C1))

    C0s = [o[6] for o in ops]
    C1s = [o[7] for o in ops]
    CLO = sum(C0s)
    STOT = CLO + sum(C1s)
    lo_base = [0, C0s[0], C0s[0] + C0s[1]]
    hi_base = [CLO, CLO + C1s[0], CLO + C1s[0] + C1s[1]]

    idxf = np.zeros(NTILE * STOT * 128, np.int16)
    ohf = np.zeros((NTILE * STOT * 128, 128), np.float16)
    rlocf = np.zeros(NTILE * STOT * 128, np.float16)
    valf = np.zeros(NTILE * STOT * 128, np.float16)
    for i, (row, col, val, t, h, pos, _, _) in enumerate(ops):
        cb = np.where(h == 0, lo_base[i], hi_base[i])
        slot = (t * STOT + cb) * 128 + pos
        idxf[slot] = (col - h * SPLIT).astype(np.int16)
        ohf[slot, row & 127] = val.astype(np.float16)
        rlocf[slot] = (row & 127).astype(np.float16)
        valf[slot] = val.astype(np.float16)

    S = tuple(zip(C0s, C1s))
    return S, STOT, idxf, ohf, rlocf, valf


def kernel(**inputs):
    x = np.asarray(inputs["x"], dtype=np.float32)
    coeffs = np.asarray(inputs["coeffs"], dtype=np.float32)
    bias = np.asarray(inputs["bias"], dtype=np.float32)

    S, STOT, idxf, ohf, rlocf, valf = _prep(inputs)

    key = (S, OH_MODE)
    if key not in _cache:
        _cache[key] = _build(S, OH_MODE)
    nc = _cache[key]

    xT = np.zeros((NVPAD, BC), np.float16)
    xT[:NV] = x.transpose(2, 0, 1).reshape(NV, BC)
    xf = np.zeros((BC, NVPAD), np.float16)
    xf[:, :NV] = x.reshape(BC, NV)
    coef16 = np.tile(
        coeffs.transpose(1, 0, 2).reshape(64, 256).astype(np.float16), (2, 1))
    bias2 = bias.reshape(64, 1).astype(np.float32)
    iota16 = np.broadcast_to(
        np.arange(128, dtype=np.float16), (128, 128)).copy()

    idxT = idxf.reshape(NTILE, STOT * 128)
    ohT = ohf.reshape(NTILE, STOT, 128, 128)
    rlocT = rlocf.reshape(NTILE, STOT, 128)
    valT = valf.reshape(NTILE, STOT, 128)

    in_maps = []
    for core in range(NCORES):
        t0, t1 = core * TPC, (core + 1) * TPC
        n = STOT * 128
        A = idxT[t0:t1]
        W = A.reshape(TPC, n // 16, 16).transpose(0, 2, 1)
        W = np.tile(W, (1, 8, 1))
        idx16 = np.ascontiguousarray(
            W.transpose(1, 0, 2).reshape(128, TPC * n // 16))
        xi = np.concatenate(
            [xf[:128, t0 * 128:t1 * 128], xf[128:, t0 * 128:t1 * 128]],
            axis=1)
        m = {
            "xg": xT,
            "xI": np.ascontiguousarray(xi),
            "idx16": idx16,
            "coef": coef16,
            "bias2": bias2,
        }
        if OH_MODE == "stream":
            m["oh"] = np.ascontiguousarray(
                ohT[t0:t1].transpose(2, 0, 1, 3).reshape(128, -1))
        else:
            m["iota16"] = iota16
            m["rloc"] = np.ascontiguousarray(
                rlocT[t0:t1].transpose(2, 0, 1).reshape(128, -1))
            m["val"] = np.ascontiguousarray(
                valT[t0:t1].transpose(2, 0, 1).reshape(128, -1))
        in_maps.append(m)

    res = run_bass_kernel_spmd(nc, in_maps, core_ids=list(range(NCORES)))
    out = np.concatenate([res.results[c]["out"] for c in range(NCORES)],
                         axis=2)
    return np.ascontiguousarray(out[:, :, :NV])


# revision 7
# speedup vs baseline: 1.3120x; 1.0065x over previous
"""MeshConv (gnn_message_passing) Bass kernel for 8 trn2 NeuronCores.

out[b,o,v] = bias[o] + sum_k coeffs[k,:,o]^T feats_k[b,v,:]
  feats_0 = x^T (identity), feats_{1,2,3} = spmm(L/EW/NS, x)

Strategy: shard output vertices across cores (row-partitioned spmm),
fp16 data path.  Edges sorted by (dest tile, col-half, col) into 128-edge
chunks; per tile one dma_gather per <=8-chunk range pulls 512B fp16 rows
of x^T (int16 indices split lo/hi around row 32768).  One-hot matrices
[edge, row_local]*val are precomputed on the host in fp16 and streamed
from HBM (no DVE work).  Per chunk the PE computes
yT[bc_half, row] += g[edge, bc_half]^T @ oh[edge, row] directly in the
transposed layout, so no PE transposes are needed; identity features are
DMA'd straight from x (natural [bc, v] layout).  Per tile pair the
per-operator coeffs hit yT (free dim 256), bias added on the Scalar
engine, output written as [o, rows] slabs per batch.
"""

import sys

sys.path.insert(0, "/opt/trn_rl_repo")

import numpy as np

import concourse.bass as bass
import concourse.bacc as bacc
import concourse.tile as tile
import concourse.mybir as mybir
from concourse.bass_utils import run_bass_kernel_spmd

NV = 40962
B = 4
C = 64
BC = B * C  # 256
NCORES = 8
NTILE = 328          # 128-row tiles, 328*128 = 41984 >= 40962
NVPAD = NTILE * 128
TPC = NTILE // NCORES  # 41 tiles per core
TPCR = TPC * 128
SPLIT = 32768        # int16 index split point
MAXD = 8             # max chunks per dma_gather call (8*128 = 1024 descs)
NQ = 4               # SWDGE queues (ucode max)

OH_MODE = "dve"      # "stream": fp16 one-hots from HBM; "dve": build on DVE

_cache = {}

OPK = ("L", "E", "N")


def _calls(CLO, STOT):
    """Static per-tile gather call list: (chunk_off, nchunks, is_hi)."""
    calls = []
    for lo, hi in ((0, CLO), (CLO, STOT)):
        a = lo
        while a < hi:
            n = min(MAXD, hi - a)
            calls.append((a, n, lo != 0))
            a += n
    return calls


def _build(S, oh_mode):
    """Build the per-core Bass program for ((CL0,CL1),(CE0,CE1),(CN0,CN1))."""
    C0s = [c0 for c0, _ in S]
    C1s = [c1 for _, c1 in S]
    CLO, CHI = sum(C0s), sum(C1s)
    STOT = CLO + CHI
    lo_base = [0, C0s[0], C0s[0] + C0s[1]]
    hi_base = [CLO, CLO + C1s[0], CLO + C1s[0] + C1s[1]]
    op_chunks = {
        k: list(range(lo_base[i], lo_base[i] + C0s[i]))
        + list(range(hi_base[i], hi_base[i] + C1s[i]))
        for i, k in enumerate(OPK)
    }
    f32 = mybir.dt.float32
    f16 = mybir.dt.float16

    nc = bacc.Bacc("TRN2", target_bir_lowering=False, debug=False,
                   num_devices=NCORES, num_swdge_queues=NQ)

    xg_d = nc.dram_tensor("xg", [NVPAD, BC], f16, kind="ExternalInput")
    xI_d = nc.dram_tensor("xI", [128, 2 * TPCR], f16, kind="ExternalInput")
    idx_d = nc.dram_tensor("idx16", [128, TPC * STOT * 8], mybir.dt.int16,
                           kind="ExternalInput")
    if oh_mode == "stream":
        oh_d = nc.dram_tensor("oh", [128, TPC * STOT * 128], f16,
                              kind="ExternalInput")
    else:
        iota_d = nc.dram_tensor("iota16", [128, 128], f16,
                                kind="ExternalInput")
        rloc_d = nc.dram_tensor("rloc", [128, TPC * STOT], f32,
                                kind="ExternalInput")
        val_d = nc.dram_tensor("val", [128, TPC * STOT], f32,
                               kind="ExternalInput")
    coef_d = nc.dram_tensor("coef", [128, 256], f16, kind="ExternalInput")
    bias_d = nc.dram_tensor("bias2", [64, 1], f32, kind="ExternalInput")
    out_d = nc.dram_tensor("out", [B, C, TPC * 128], f32,
                           kind="ExternalOutput")

    calls = _calls(CLO, STOT)

    with tile.TileContext(nc) as tc:
        with (
            tc.tile_pool(name="const", bufs=1) as cpool,
            tc.tile_pool(name="meta", bufs=1) as mpool,
            tc.tile_pool(name="g", bufs=2) as gpool,
            tc.tile_pool(name="oh", bufs=2) as ohpool,
            tc.tile_pool(name="yt", bufs=2) as ytpool,
            tc.tile_pool(name="os", bufs=2) as ospool,
            tc.tile_pool(name="py", bufs=1, space="PSUM") as pypool,
            tc.tile_pool(name="po", bufs=2, space="PSUM") as popool,
        ):
            # constants
            coef_t = cpool.tile([128, 256], f16)
            nc.sync.dma_start(coef_t[:], coef_d.ap()[:])
            bias_t = cpool.tile([64, 1], f32)
            nc.sync.dma_start(bias_t[:], bias_d.ap()[:])
            idx_t = mpool.tile([128, TPC * STOT * 8], mybir.dt.int16)
            nc.sync.dma_start(idx_t[:], idx_d.ap()[:])
            if oh_mode != "stream":
                iota_t = cpool.tile([128, 128], f16)
                nc.sync.dma_start(iota_t[:], iota_d.ap()[:])
                rloc_t = mpool.tile([128, TPC * STOT], f32)
                nc.sync.dma_start(rloc_t[:], rloc_d.ap()[:])
                val_t = mpool.tile([128, TPC * STOT], f32)
                nc.sync.dma_start(val_t[:], val_d.ap()[:])

            yts = {}  # (k, hb) -> staging tile [128, 256] f16 per tile pair
            qn = 0

            for t in range(TPC):
                pair_off = (t % 2) * 128
                is_pair_start = t % 2 == 0
                is_orphan = t == TPC - 1 and is_pair_start

                g_t = gpool.tile([128, STOT * BC], f16, tag="g")
                ib = t * STOT * 8
                for coff, nch, hi in calls:
                    src = xg_d.ap()[SPLIT:, :] if hi else xg_d.ap()[:SPLIT, :]
                    nc.gpsimd.dma_gather(
                        out_ap=g_t[:, coff * BC:(coff + nch) * BC]
                        .rearrange("p (j f) -> p j f", f=BC),
                        in_ap=src,
                        idxs_ap=idx_t[:, ib + coff * 8:ib + (coff + nch) * 8],
                        num_idxs=nch * 128,
                        num_idxs_reg=nch * 128,
                        elem_size=BC,
                        queue_num=qn % NQ,
                    )
                    qn += 1

                oh_t = ohpool.tile([128, STOT * 128], f16, tag="oh")
                if oh_mode == "stream":
                    nc.sync.dma_start(
                        oh_t[:],
                        oh_d.ap()[:, t * STOT * 128:(t + 1) * STOT * 128])
                else:
                    for cj in range(STOT):
                        mcol = t * STOT + cj
                        nc.vector.tensor_scalar(
                            out=oh_t[:, cj * 128:(cj + 1) * 128],
                            in0=iota_t[:],
                            scalar1=rloc_t[:, mcol:mcol + 1],
                            scalar2=val_t[:, mcol:mcol + 1],
                            op0=mybir.AluOpType.is_equal,
                            op1=mybir.AluOpType.mult,
                        )

                if is_pair_start:
                    for k in ("I",) + OPK:
                        for hb in (0, 1):
                            yts[(k, hb)] = ytpool.tile(
                                [128, 256], f16, tag=f"yt{k}{hb}",
                                name=f"yt{k}{hb}_{t}")
                            if is_orphan:
                                nc.vector.memset(
                                    yts[(k, hb)][:].bitcast(f32), 0.0)

                # identity features straight from x (natural layout)
                for hb in (0, 1):
                    nc.sync.dma_start(
                        yts[("I", hb)][:, pair_off:pair_off + 128],
                        xI_d.ap()[:, hb * TPCR + t * 128:
                                  hb * TPCR + (t + 1) * 128])

                # spmm: yT[bc_half, row] += g[e, bc_half]^T @ oh[e, row]
                for k in OPK:
                    chunks = op_chunks[k]
                    for hb in (0, 1):
                        py_t = pypool.tile([128, 128], f32, tag=f"py{k}{hb}")
                        for ci, cj in enumerate(chunks):
                            nc.tensor.matmul(
                                py_t[:],
                                g_t[:, cj * BC + hb * 128:
                                    cj * BC + hb * 128 + 128],
                                oh_t[:, cj * 128:(cj + 1) * 128],
                                start=(ci == 0),
                                stop=(ci == len(chunks) - 1),
                            )
                        nc.scalar.activation(
                            yts[(k, hb)][:, pair_off:pair_off + 128], py_t[:],
                            mybir.ActivationFunctionType.Copy)

                # coeffs matmuls on completed pair
                if not is_pair_start or is_orphan:
                    r0 = (t - 1 if not is_pair_start else t) * 128
                    ncols = 128 if is_orphan else 256
                    for b in range(B):
                        po_t = popool.tile([64, 256], f32, tag="po",
                                           name=f"po{b}_{t}")
                        for ki, k in enumerate(("I",) + OPK):
                            p0 = (b % 2) * 64
                            rhs = yts[(k, b // 2)][p0:p0 + 64, :]
                            nc.tensor.matmul(
                                po_t[:],
                                coef_t[p0:p0 + 64, ki * 64:(ki + 1) * 64],
                                rhs,
                                start=(ki == 0),
                                stop=(ki == 3),
                            )
                        os_t = ospool.tile([64, 256], f32, tag="os",
                                           name=f"os{b}_{t}")
                        nc.scalar.activation(
                            os_t[:], po_t[:],
                            mybir.ActivationFunctionType.Identity,
                            bias=bias_t[:, :1])
                        nc.sync.dma_start(
                            out_d.ap()[b:b + 1, :, r0:r0 + ncols]
                            .rearrange("b o r -> (b o) r"),
                            os_t[:, :ncols])

    nc.compile()
    return nc


def _prep(inputs):
    """Sort edges by (tile, col-half, col); compute global chunk counts and
    fill flat slot arrays (idx, one-hot / rloc+val)."""
    ops = []
    for name in ("L", "EW", "NS"):
        row = np.asarray(inputs[f"{name}_row"]).astype(np.int64)
        col = np.asarray(inputs[f"{name}_col"]).astype(np.int64)
        val = np.asarray(inputs[f"{name}_val"]).astype(np.float32)
        t = row >> 7
        h = (col >= SPLIT).astype(np.int64)
        order = np.lexsort((col, h, t))
        row, col, val, t, h = (a[order] for a in (row, col, val, t, h))
        grp = t * 2 + h
        counts = np.bincount(grp, minlength=NTILE * 2)
        starts = np.zeros(NTILE * 2, np.int64)
        starts[1:] = np.cumsum(counts)[:-1]
        pos = np.arange(len(row)) - starts[grp]
        C0 = int(np.ceil(max(int(counts[0::2].max()), 1) / 128))
        C1 = int(np.ceil(max(int(counts[1::2].max()), 1) / 128))
        ops.append((row, col, val, t, h, pos, C0, C1))

    C0s = [o[6] for o in ops]
    C1s = [o[7] for o in ops]
    CLO = sum(C0s)
    STOT = CLO + sum(C1s)
    lo_base = [0, C0s[0], C0s[0] + C0s[1]]
    hi_base = [CLO, CLO + C1s[0], CLO + C1s[0] + C1s[1]]

    idxf = np.zeros(NTILE * STOT * 128, np.int16)
    ohf = np.zeros((NTILE * STOT * 128, 128), np.float16)
    rlocf = np.zeros(NTILE * STOT * 128, np.float32)
    valf = np.zeros(NTILE * STOT * 128, np.float32)
    for i, (row, col, val, t, h, pos, _, _) in enumerate(ops):
        cb = np.where(h == 0, lo_base[i], hi_base[i])
        slot = (t * STOT + cb) * 128 + pos
        idxf[slot] = (col - h * SPLIT).astype(np.int16)
        ohf[slot, row & 127] = val.astype(np.float16)
        rlocf[slot] = (row & 127).astype(np.float32)
        valf[slot] = val.astype(np.float32)

    S = tuple(zip(C0s, C1s))
    return S, STOT, idxf, ohf, rlocf, valf


def kernel(**inputs):
    x = np.asarray(inputs["x"], dtype=np.float32)
    coeffs = np.asarray(inputs["coeffs"], dtype=np.float32)
    bias = np.asarray(inputs["bias"], dtype=np.float32)

    S, STOT, idxf, ohf, rlocf, valf = _prep(inputs)

    key = (S, OH_MODE)
    if key not in _cache:
        _cache[key] = _build(S, OH_MODE)
    nc = _cache[key]

    xT = np.zeros((NVPAD, BC), np.float16)
    xT[:NV] = x.transpose(2, 0, 1).reshape(NV, BC)
    xf = np.zeros((BC, NVPAD), np.float16)
    xf[:, :NV] = x.reshape(BC, NV)
    coef16 = np.tile(
        coeffs.transpose(1, 0, 2).reshape(64, 256).astype(np.float16), (2, 1))
    bias2 = bias.reshape(64, 1).astype(np.float32)
    iota16 = np.broadcast_to(
        np.arange(128, dtype=np.float16), (128, 128)).copy()

    idxT = idxf.reshape(NTILE, STOT * 128)
    ohT = ohf.reshape(NTILE, STOT, 128, 128)
    rlocT = rlocf.reshape(NTILE, STOT, 128)
    valT = valf.reshape(NTILE, STOT, 128)

    in_maps = []
    for core in range(NCORES):
        t0, t1 = core * TPC, (core + 1) * TPC
        n = STOT * 128
        A = idxT[t0:t1]
        W = A.reshape(TPC, n // 16, 16).transpose(0, 2, 1)
        W = np.tile(W, (1, 8, 1))
        idx16 = np.ascontiguousarray(
            W.transpose(1, 0, 2).reshape(128, TPC * n // 16))
        xi = np.concatenate(
            [xf[:128, t0 * 128:t1 * 128], xf[128:, t0 * 128:t1 * 128]],
            axis=1)
        m = {
            "xg": xT,
            "xI": np.ascontiguousarray(xi),
            "idx16": idx16,
            "coef": coef16,
            "bias2": bias2,
        }
        if OH_MODE == "stream":
            m["oh"] = np.ascontiguousarray(
                ohT[t0:t1].transpose(2, 0, 1, 3).reshape(128, -1))
        else:
            m["iota16"] = iota16
            m["rloc"] = np.ascontiguousarray(
                rlocT[t0:t1].transpose(2, 0, 1).reshape(128, -1))
            m["val"] = np.ascontiguousarray(
                valT[t0:t1].transpose(2, 0, 1).reshape(128, -1))
        in_maps.append(m)

    res = run_bass_kernel_spmd(nc, in_maps, core_ids=list(range(NCORES)))
    out = np.concatenate([res.results[c]["out"] for c in range(NCORES)],
                         axis=2)
    return np.ascontiguousarray(out[:, :, :NV])


# revision 10
# speedup vs baseline: 1.6246x; 1.2383x over previous
"""MeshConv (gnn_message_passing) Bass kernel for 8 trn2 NeuronCores.

out[b,o,v] = bias[o] + sum_k coeffs[k,:,o]^T feats_k[b,v,:]
  feats_0 = x^T (identity), feats_{1,2,3} = spmm(L/EW/NS, x)

Strategy: shard output vertices across cores (row-partitioned spmm),
fp16 data path.  Edges sorted by (dest tile, col-half, col) into 128-edge
chunks; per tile one dma_gather per <=8-chunk range pulls 512B fp16 rows
of x^T (int16 indices split lo/hi around row 32768).  One-hot matrices
[edge, row_local]*val are precomputed on the host in fp16 and streamed
from HBM (no DVE work).  Per chunk the PE computes
yT[bc_half, row] += g[edge, bc_half]^T @ oh[edge, row] directly in the
transposed layout, so no PE transposes are needed; identity features are
DMA'd straight from x (natural [bc, v] layout).  Per tile pair the
per-operator coeffs hit yT (free dim 256), bias added on the Scalar
engine, output written as [o, rows] slabs per batch.
"""

import sys

sys.path.insert(0, "/opt/trn_rl_repo")

import numpy as np

import concourse.bass as bass
import concourse.bacc as bacc
import concourse.tile as tile
import concourse.mybir as mybir
from concourse.bass_utils import run_bass_kernel_spmd

NV = 40962
B = 4
C = 64
BC = B * C  # 256
NCORES = 8
NTILE = 328          # 128-row tiles, 328*128 = 41984 >= 40962
NVPAD = NTILE * 128
TPC = NTILE // NCORES  # 41 tiles per core
TPCR = TPC * 128
SPLIT = 32768        # int16 index split point
MAXD = 4             # max chunks per gather call (512 descs: 2 fit the ring)
NQ = 4               # SWDGE queues (ucode max)

OH_MODE = "hybrid"   # stream most one-hots from HBM, build every DVE_EVERY-th on DVE
DVE_EVERY = 3        # chunk cj goes to DVE when cj % DVE_EVERY == 0

_cache = {}

OPK = ("L", "E", "N")


def _calls(CLO, STOT):
    """Static per-tile gather call list: (chunk_off, nchunks, is_hi)."""
    calls = []
    for lo, hi in ((0, CLO), (CLO, STOT)):
        a = lo
        while a < hi:
            n = min(MAXD, hi - a)
            calls.append((a, n, lo != 0))
            a += n
    return calls


def _build(S, oh_mode):
    """Build the per-core Bass program for ((CL0,CL1),(CE0,CE1),(CN0,CN1))."""
    C0s = [c0 for c0, _ in S]
    C1s = [c1 for _, c1 in S]
    CLO, CHI = sum(C0s), sum(C1s)
    STOT = CLO + CHI
    lo_base = [0, C0s[0], C0s[0] + C0s[1]]
    hi_base = [CLO, CLO + C1s[0], CLO + C1s[0] + C1s[1]]
    op_chunks = {
        k: list(range(lo_base[i], lo_base[i] + C0s[i]))
        + list(range(hi_base[i], hi_base[i] + C1s[i]))
        for i, k in enumerate(OPK)
    }
    f32 = mybir.dt.float32
    f16 = mybir.dt.float16

    nc = bacc.Bacc("TRN2", target_bir_lowering=False, debug=False,
                   num_devices=NCORES, num_swdge_queues=NQ)

    xg_d = nc.dram_tensor("xg", [NVPAD, BC], f16, kind="ExternalInput")
    xI_d = nc.dram_tensor("xI", [128, 2 * TPCR], f16, kind="ExternalInput")
    idx_d = nc.dram_tensor("idx16", [128, TPC * STOT * 8], mybir.dt.int16,
                           kind="ExternalInput")
    DCH = [cj for cj in range(STOT) if cj % DVE_EVERY == 0]
    SCH = [cj for cj in range(STOT) if cj % DVE_EVERY != 0]
    ND, NS = len(DCH), len(SCH)
    chpos = {}
    for i, cj in enumerate(DCH):
        chpos[cj] = ("v", i)
    for i, cj in enumerate(SCH):
        chpos[cj] = ("s", i)
    oh_d = nc.dram_tensor("oh", [128, TPC * NS * 128], f16,
                          kind="ExternalInput")
    iota_d = nc.dram_tensor("iota16", [128, 128], f16, kind="ExternalInput")
    rloc_d = nc.dram_tensor("rloc", [128, TPC * ND], f32,
                            kind="ExternalInput")
    val_d = nc.dram_tensor("val", [128, TPC * ND], f32,
                           kind="ExternalInput")
    coef_d = nc.dram_tensor("coef", [128, 256], f16, kind="ExternalInput")
    bias_d = nc.dram_tensor("bias2", [64, 1], f32, kind="ExternalInput")
    out_d = nc.dram_tensor("out", [B, C, TPC * 128], f32,
                           kind="ExternalOutput")

    calls = _calls(CLO, STOT)

    with tile.TileContext(nc) as tc:
        with (
            tc.tile_pool(name="const", bufs=1) as cpool,
            tc.tile_pool(name="meta", bufs=1) as mpool,
            tc.tile_pool(name="g", bufs=2) as gpool,
            tc.tile_pool(name="oh", bufs=2) as ohpool,
            tc.tile_pool(name="yt", bufs=2) as ytpool,
            tc.tile_pool(name="os", bufs=2) as ospool,
            tc.tile_pool(name="py", bufs=1, space="PSUM") as pypool,
            tc.tile_pool(name="po", bufs=2, space="PSUM") as popool,
        ):
            # constants
            coef_t = cpool.tile([128, 256], f16)
            nc.sync.dma_start(coef_t[:], coef_d.ap()[:])
            bias_t = cpool.tile([64, 1], f32)
            nc.sync.dma_start(bias_t[:], bias_d.ap()[:])
            idx_t = mpool.tile([128, TPC * STOT * 8], mybir.dt.int16)
            nc.sync.dma_start(idx_t[:], idx_d.ap()[:])
            iota_t = cpool.tile([128, 128], f16)
            nc.sync.dma_start(iota_t[:], iota_d.ap()[:])
            rloc_t = mpool.tile([128, TPC * ND], f32)
            nc.sync.dma_start(rloc_t[:], rloc_d.ap()[:])
            val_t = mpool.tile([128, TPC * ND], f32)
            nc.sync.dma_start(val_t[:], val_d.ap()[:])

            yts = {}  # (k, hb) -> staging tile [128, 256] f16 per tile pair
            qn = 0

            for t in range(TPC):
                pair_off = (t % 2) * 128
                is_pair_start = t % 2 == 0
                is_orphan = t == TPC - 1 and is_pair_start

                g_t = gpool.tile([128, STOT * BC], f16, tag="g")
                ib = t * STOT * 8
                for coff, nch, hi in calls:
                    src = xg_d.ap()[SPLIT:, :] if hi else xg_d.ap()[:SPLIT, :]
                    nc.gpsimd.dma_gather(
                        out_ap=g_t[:, coff * BC:(coff + nch) * BC]
                        .rearrange("p (j f) -> p j f", f=BC),
                        in_ap=src,
                        idxs_ap=idx_t[:, ib + coff * 8:ib + (coff + nch) * 8],
                        num_idxs=nch * 128,
                        num_idxs_reg=nch * 128,
                        elem_size=BC,
                        queue_num=qn % NQ,
                    )
                    qn += 1

                oh_s = ohpool.tile([128, NS * 128], f16, tag="ohs")
                nc.sync.dma_start(
                    oh_s[:], oh_d.ap()[:, t * NS * 128:(t + 1) * NS * 128])
                oh_v = ohpool.tile([128, ND * 128], f16, tag="ohv")
                for i in range(ND):
                    mcol = t * ND + i
                    nc.vector.tensor_scalar(
                        out=oh_v[:, i * 128:(i + 1) * 128],
                        in0=iota_t[:],
                        scalar1=rloc_t[:, mcol:mcol + 1],
                        scalar2=val_t[:, mcol:mcol + 1],
                        op0=mybir.AluOpType.is_equal,
                        op1=mybir.AluOpType.mult,
                    )

                def oh_ap(cj):
                    kind, i = chpos[cj]
                    tl = oh_v if kind == "v" else oh_s
                    return tl[:, i * 128:(i + 1) * 128]

                if is_pair_start:
                    for k in ("I",) + OPK:
                        for hb in (0, 1):
                            yts[(k, hb)] = ytpool.tile(
                                [128, 256], f16, tag=f"yt{k}{hb}",
                                name=f"yt{k}{hb}_{t}")
                            if is_orphan:
                                nc.vector.memset(
                                    yts[(k, hb)][:].bitcast(f32), 0.0)

                # identity features straight from x (natural layout)
                for hb in (0, 1):
                    nc.sync.dma_start(
                        yts[("I", hb)][:, pair_off:pair_off + 128],
                        xI_d.ap()[:, hb * TPCR + t * 128:
                                  hb * TPCR + (t + 1) * 128])

                # spmm: yT[bc_half, row] += g[e, bc_half]^T @ oh[e, row]
                for k in OPK:
                    chunks = op_chunks[k]
                    for hb in (0, 1):
                        py_t = pypool.tile([128, 128], f32, tag=f"py{k}{hb}")
                        for ci, cj in enumerate(chunks):
                            nc.tensor.matmul(
                                py_t[:],
                                g_t[:, cj * BC + hb * 128:
                                    cj * BC + hb * 128 + 128],
                                oh_ap(cj),
                                start=(ci == 0),
                                stop=(ci == len(chunks) - 1),
                            )
                        nc.scalar.activation(
                            yts[(k, hb)][:, pair_off:pair_off + 128], py_t[:],
                            mybir.ActivationFunctionType.Copy)

                # coeffs matmuls on completed pair
                if not is_pair_start or is_orphan:
                    r0 = (t - 1 if not is_pair_start else t) * 128
                    ncols = 128 if is_orphan else 256
                    for b in range(B):
                        po_t = popool.tile([64, 256], f32, tag="po",
                                           name=f"po{b}_{t}")
                        for ki, k in enumerate(("I",) + OPK):
                            p0 = (b % 2) * 64
                            rhs = yts[(k, b // 2)][p0:p0 + 64, :]
                            nc.tensor.matmul(
                                po_t[:],
                                coef_t[p0:p0 + 64, ki * 64:(ki + 1) * 64],
                                rhs,
                                start=(ki == 0),
                                stop=(ki == 3),
                            )
                        os_t = ospool.tile([64, 256], f32, tag="os",
                                           name=f"os{b}_{t}")
                        nc.scalar.activation(
                            os_t[:], po_t[:],
                            mybir.ActivationFunctionType.Identity,
                            bias=bias_t[:, :1])
                        nc.sync.dma_start(
                            out_d.ap()[b:b + 1, :, r0:r0 + ncols]
                            .rearrange("b o r -> (b o) r"),
                            os_t[:, :ncols])

    nc.compile()
    return nc


def _prep(inputs):
    """Sort edges by (tile, col-half, col); compute global chunk counts and
    fill flat slot arrays (idx, one-hot / rloc+val)."""
    ops = []
    for name in ("L", "EW", "NS"):
        row = np.asarray(inputs[f"{name}_row"]).astype(np.int64)
        col = np.asarray(inputs[f"{name}_col"]).astype(np.int64)
        val = np.asarray(inputs[f"{name}_val"]).astype(np.float32)
        t = row >> 7
        h = (col >= SPLIT).astype(np.int64)
        order = np.lexsort((col, h, t))
        row, col, val, t, h = (a[order] for a in (row, col, val, t, h))
        grp = t * 2 + h
        counts = np.bincount(grp, minlength=NTILE * 2)
        starts = np.zeros(NTILE * 2, np.int64)
        starts[1:] = np.cumsum(counts)[:-1]
        pos = np.arange(len(row)) - starts[grp]
        C0 = int(np.ceil(max(int(counts[0::2].max()), 1) / 128))
        C1 = int(np.ceil(max(int(counts[1::2].max()), 1) / 128))
        ops.append((row, col, val, t, h, pos, C0, C1))

    C0s = [o[6] for o in ops]
    C1s = [o[7] for o in ops]
    CLO = sum(C0s)
    STOT = CLO + sum(C1s)
    lo_base = [0, C0s[0], C0s[0] + C0s[1]]
    hi_base = [CLO, CLO + C1s[0], CLO + C1s[0] + C1s[1]]

    idxf = np.zeros(NTILE * STOT * 128, np.int16)
    ohf = np.zeros((NTILE * STOT * 128, 128), np.float16)
    rlocf = np.zeros(NTILE * STOT * 128, np.float32)
    valf = np.zeros(NTILE * STOT * 128, np.float32)
    for i, (row, col, val, t, h, pos, _, _) in enumerate(ops):
        cb = np.where(h == 0, lo_base[i], hi_base[i])
        slot = (t * STOT + cb) * 128 + pos
        idxf[slot] = (col - h * SPLIT).astype(np.int16)
        ohf[slot, row & 127] = val.astype(np.float16)
        rlocf[slot] = (row & 127).astype(np.float32)
        valf[slot] = val.astype(np.float32)

    S = tuple(zip(C0s, C1s))
    return S, STOT, idxf, ohf, rlocf, valf


def kernel(**inputs):
    x = np.asarray(inputs["x"], dtype=np.float32)
    coeffs = np.asarray(inputs["coeffs"], dtype=np.float32)
    bias = np.asarray(inputs["bias"], dtype=np.float32)

    S, STOT, idxf, ohf, rlocf, valf = _prep(inputs)

    key = (S, OH_MODE)
    if key not in _cache:
        _cache[key] = _build(S, OH_MODE)
    nc = _cache[key]

    xT = np.zeros((NVPAD, BC), np.float16)
    xT[:NV] = x.transpose(2, 0, 1).reshape(NV, BC)
    xf = np.zeros((BC, NVPAD), np.float16)
    xf[:, :NV] = x.reshape(BC, NV)
    coef16 = np.tile(
        coeffs.transpose(1, 0, 2).reshape(64, 256).astype(np.float16), (2, 1))
    bias2 = bias.reshape(64, 1).astype(np.float32)
    iota16 = np.broadcast_to(
        np.arange(128, dtype=np.float16), (128, 128)).copy()

    idxT = idxf.reshape(NTILE, STOT * 128)
    ohT = ohf.reshape(NTILE, STOT, 128, 128)
    rlocT = rlocf.reshape(NTILE, STOT, 128)
    valT = valf.reshape(NTILE, STOT, 128)
    DCH = [cj for cj in range(STOT) if cj % DVE_EVERY == 0]
    SCH = [cj for cj in range(STOT) if cj % DVE_EVERY != 0]

    in_maps = []
    for core in range(NCORES):
        t0, t1 = core * TPC, (core + 1) * TPC
        n = STOT * 128
        A = idxT[t0:t1]
        W = A.reshape(TPC, n // 16, 16).transpose(0, 2, 1)
        W = np.tile(W, (1, 8, 1))
        idx16 = np.ascontiguousarray(
            W.transpose(1, 0, 2).reshape(128, TPC * n // 16))
        xi = np.concatenate(
            [xf[:128, t0 * 128:t1 * 128], xf[128:, t0 * 128:t1 * 128]],
            axis=1)
        m = {
            "xg": xT,
            "xI": np.ascontiguousarray(xi),
            "idx16": idx16,
            "coef": coef16,
            "bias2": bias2,
            "iota16": iota16,
            "oh": np.ascontiguousarray(
                ohT[t0:t1][:, SCH].transpose(2, 0, 1, 3).reshape(128, -1)),
            "rloc": np.ascontiguousarray(
                rlocT[t0:t1][:, DCH].transpose(2, 0, 1).reshape(128, -1)),
            "val": np.ascontiguousarray(
                valT[t0:t1][:, DCH].transpose(2, 0, 1).reshape(128, -1)),
        }
        in_maps.append(m)

    res = run_bass_kernel_spmd(nc, in_maps, core_ids=list(range(NCORES)))
    out = np.concatenate([res.results[c]["out"] for c in range(NCORES)],
                         axis=2)
    return np.ascontiguousarray(out[:, :, :NV])


# revision 16
# speedup vs baseline: 1.7355x; 1.0683x over previous
"""MeshConv (gnn_message_passing) Bass kernel for 8 trn2 NeuronCores.

out[b,o,v] = bias[o] + sum_k coeffs[k,:,o]^T feats_k[b,v,:]
  feats_0 = x^T (identity), feats_{1,2,3} = spmm(L/EW/NS, x)

Strategy: shard output vertices across cores (row-partitioned spmm),
fp16 data path.  Edges sorted by (dest tile, col-half, col) into 128-edge
chunks; per tile one dma_gather per <=8-chunk range pulls 512B fp16 rows
of x^T (int16 indices split lo/hi around row 32768).  One-hot matrices
[edge, row_local]*val are precomputed on the host in fp16 and streamed
from HBM (no DVE work).  Per chunk the PE computes
yT[bc_half, row] += g[edge, bc_half]^T @ oh[edge, row] directly in the
transposed layout, so no PE transposes are needed; identity features are
DMA'd straight from x (natural [bc, v] layout).  Per tile pair the
per-operator coeffs hit yT (free dim 256), bias added on the Scalar
engine, output written as [o, rows] slabs per batch.
"""

import sys

sys.path.insert(0, "/opt/trn_rl_repo")

import numpy as np

import concourse.bass as bass
import concourse.bacc as bacc
import concourse.tile as tile
import concourse.mybir as mybir
from concourse.bass_utils import run_bass_kernel_spmd

NV = 40962
B = 4
C = 64
BC = B * C  # 256
NCORES = 8
NTILE = 328          # 128-row tiles, 328*128 = 41984 >= 40962
NVPAD = NTILE * 128
TPC = NTILE // NCORES  # 41 tiles per core
TPCR = TPC * 128
SPLIT = 32768        # int16 index split point
MAXD = 4             # max chunks per gather call (512 descs: 2 fit the ring)
NQ = 4               # SWDGE queues (ucode max)

OH_MODE = "hybrid"   # stream most one-hots from HBM, build every DVE_EVERY-th on DVE
DVE_EVERY = 4        # chunk cj goes to DVE when cj % DVE_EVERY == 0
TRIM = "on"          # "on": runtime-trimmed counts; "regfull": registers but
                     # full counts (debug); "off": static num_idxs

_cache = {}

OPK = ("L", "E", "N")


def _pieces(S):
    """Group-aligned gather pieces: (chunk_off, nchunks, is_hi, grp_start).

    Each piece stays inside one (op, half) group so slot padding is always
    trailing within the piece and can be trimmed via num_idxs_reg."""
    C0s = [c0 for c0, _ in S]
    C1s = [c1 for _, c1 in S]
    groups = [(C0s[0], 0), (C0s[1], 0), (C0s[2], 0),
              (C1s[0], 1), (C1s[1], 1), (C1s[2], 1)]
    pieces = []
    base = 0
    for Cg, hi in groups:
        a = 0
        while a < Cg:
            n = min(MAXD, Cg - a)
            pieces.append((base + a, n, hi, base))
            a += n
        base += Cg
    return pieces


def _build(S, oh_mode):
    """Build the per-core Bass program for ((CL0,CL1),(CE0,CE1),(CN0,CN1))."""
    C0s = [c0 for c0, _ in S]
    C1s = [c1 for _, c1 in S]
    CLO, CHI = sum(C0s), sum(C1s)
    STOT = CLO + CHI
    lo_base = [0, C0s[0], C0s[0] + C0s[1]]
    hi_base = [CLO, CLO + C1s[0], CLO + C1s[0] + C1s[1]]
    op_chunks = {
        k: list(range(lo_base[i], lo_base[i] + C0s[i]))
        + list(range(hi_base[i], hi_base[i] + C1s[i]))
        for i, k in enumerate(OPK)
    }
    f32 = mybir.dt.float32
    f16 = mybir.dt.float16

    nc = bacc.Bacc("TRN2", target_bir_lowering=False, debug=False,
                   num_devices=NCORES, num_swdge_queues=NQ)

    xg_d = nc.dram_tensor("xg", [NVPAD, BC], f16, kind="ExternalInput")
    xI_d = nc.dram_tensor("xI", [128, 2 * TPCR], f16, kind="ExternalInput")
    idx_d = nc.dram_tensor("idx16", [128, TPC * STOT * 8], mybir.dt.int16,
                           kind="ExternalInput")
    DCH = [cj for cj in range(STOT) if cj % DVE_EVERY == 0]
    SCH = [cj for cj in range(STOT) if cj % DVE_EVERY != 0]
    ND, NS = len(DCH), len(SCH)
    chpos = {}
    for i, cj in enumerate(DCH):
        chpos[cj] = ("v", i)
    for i, cj in enumerate(SCH):
        chpos[cj] = ("s", i)
    oh_d = nc.dram_tensor("oh", [128, TPC * NS * 128], f16,
                          kind="ExternalInput")
    iota_d = nc.dram_tensor("iota16", [128, 128], f16, kind="ExternalInput")
    rloc_d = nc.dram_tensor("rloc", [128, TPC * ND], f32,
                            kind="ExternalInput")
    val_d = nc.dram_tensor("val", [128, TPC * ND], f32,
                           kind="ExternalInput")
    cnt_d = nc.dram_tensor("cnt", [1, TPC * len(_pieces(S))], mybir.dt.int32,
                           kind="ExternalInput")
    coef_d = nc.dram_tensor("coef", [128, 256], f16, kind="ExternalInput")
    bias_d = nc.dram_tensor("bias2", [64, 1], f32, kind="ExternalInput")
    out_d = nc.dram_tensor("out", [B, C, TPC * 128], f32,
                           kind="ExternalOutput")

    pieces = _pieces(S)
    NP = len(pieces)

    with tile.TileContext(nc) as tc:
        with (
            tc.tile_pool(name="const", bufs=1) as cpool,
            tc.tile_pool(name="meta", bufs=1) as mpool,
            tc.tile_pool(name="g", bufs=2) as gpool,
            tc.tile_pool(name="oh", bufs=2) as ohpool,
            tc.tile_pool(name="yt", bufs=2) as ytpool,
            tc.tile_pool(name="os", bufs=2) as ospool,
            tc.tile_pool(name="py", bufs=1, space="PSUM") as pypool,
            tc.tile_pool(name="po", bufs=2, space="PSUM") as popool,
        ):
            # constants
            coef_t = cpool.tile([128, 256], f16)
            nc.sync.dma_start(coef_t[:], coef_d.ap()[:])
            bias_t = cpool.tile([64, 1], f32)
            nc.sync.dma_start(bias_t[:], bias_d.ap()[:])
            idx_t = mpool.tile([128, TPC * STOT * 8], mybir.dt.int16)
            nc.sync.dma_start(idx_t[:], idx_d.ap()[:])
            cnt_t = mpool.tile([1, TPC * NP], mybir.dt.int32)
            nc.sync.dma_start(cnt_t[:], cnt_d.ap()[:])
            iota_t = cpool.tile([128, 128], f16)
            nc.sync.dma_start(iota_t[:], iota_d.ap()[:])
            rloc_t = mpool.tile([128, TPC * ND], f32)
            nc.sync.dma_start(rloc_t[:], rloc_d.ap()[:])
            val_t = mpool.tile([128, TPC * ND], f32)
            nc.sync.dma_start(val_t[:], val_d.ap()[:])

            yts = {}  # (k, hb) -> staging tile [128, 256] f16 per tile pair
            qn = 0
            creg = nc.gpsimd.alloc_register("cnt_reg")

            for t in range(TPC):
                pair_off = (t % 2) * 128
                is_pair_start = t % 2 == 0
                is_orphan = t == TPC - 1 and is_pair_start

                g_t = gpool.tile([128, STOT * BC], f16, tag="g")
                ib = t * STOT * 8
                for pi, (coff, nch, hi, _) in enumerate(pieces):
                    src = xg_d.ap()[SPLIT:, :] if hi else xg_d.ap()[:SPLIT, :]
                    if t < 2 or TRIM == "off":
                        nreg = nch * 128
                    else:
                        ci = t * NP + pi
                        nc.gpsimd.reg_load(creg, cnt_t[0:1, ci:ci + 1])
                        nreg = creg
                    nc.gpsimd.dma_gather(
                        out_ap=g_t[:, coff * BC:(coff + nch) * BC]
                        .rearrange("p (j f) -> p j f", f=BC),
                        in_ap=src,
                        idxs_ap=idx_t[:, ib + coff * 8:ib + (coff + nch) * 8],
                        num_idxs=nch * 128,
                        num_idxs_reg=nreg,
                        elem_size=BC,
                        queue_num=qn % NQ,
                    )
                    qn += 1

                oh_s = ohpool.tile([128, NS * 128], f16, tag="ohs")
                nc.sync.dma_start(
                    oh_s[:], oh_d.ap()[:, t * NS * 128:(t + 1) * NS * 128])
                oh_v = ohpool.tile([128, ND * 128], f16, tag="ohv")
                for i in range(ND):
                    mcol = t * ND + i
                    nc.vector.tensor_scalar(
                        out=oh_v[:, i * 128:(i + 1) * 128],
                        in0=iota_t[:],
                        scalar1=rloc_t[:, mcol:mcol + 1],
                        scalar2=val_t[:, mcol:mcol + 1],
                        op0=mybir.AluOpType.is_equal,
                        op1=mybir.AluOpType.mult,
                    )

                def oh_ap(cj):
                    kind, i = chpos[cj]
                    tl = oh_v if kind == "v" else oh_s
                    return tl[:, i * 128:(i + 1) * 128]

                if is_pair_start:
                    for k in ("I",) + OPK:
                        for hb in (0, 1):
                            yts[(k, hb)] = ytpool.tile(
                                [128, 256], f16, tag=f"yt{k}{hb}",
                                name=f"yt{k}{hb}_{t}")
                            if is_orphan:
                                nc.vector.memset(
                                    yts[(k, hb)][:].bitcast(f32), 0.0)

                # identity features straight from x (natural layout)
                for hb in (0, 1):
                    nc.sync.dma_start(
                        yts[("I", hb)][:, pair_off:pair_off + 128],
                        xI_d.ap()[:, hb * TPCR + t * 128:
                                  hb * TPCR + (t + 1) * 128])

                # spmm: yT[bc_half, row] += g[e, bc_half]^T @ oh[e, row]
                for k in OPK:
                    chunks = op_chunks[k]
                    for hb in (0, 1):
                        py_t = pypool.tile([128, 128], f32, tag=f"py{k}{hb}")
                        for ci, cj in enumerate(chunks):
                            nc.tensor.matmul(
                                py_t[:],
                                g_t[:, cj * BC + hb * 128:
                                    cj * BC + hb * 128 + 128],
                                oh_ap(cj),
                                start=(ci == 0),
                                stop=(ci == len(chunks) - 1),
                            )
                        nc.scalar.activation(
                            yts[(k, hb)][:, pair_off:pair_off + 128], py_t[:],
                            mybir.ActivationFunctionType.Copy)

                # coeffs matmuls on completed pair
                if not is_pair_start or is_orphan:
                    r0 = (t - 1 if not is_pair_start else t) * 128
                    ncols = 128 if is_orphan else 256
                    for b in range(B):
                        po_t = popool.tile([64, 256], f32, tag="po",
                                           name=f"po{b}_{t}")
                        for ki, k in enumerate(("I",) + OPK):
                            p0 = (b % 2) * 64
                            rhs = yts[(k, b // 2)][p0:p0 + 64, :]
                            nc.tensor.matmul(
                                po_t[:],
                                coef_t[p0:p0 + 64, ki * 64:(ki + 1) * 64],
                                rhs,
                                start=(ki == 0),
                                stop=(ki == 3),
                            )
                        os_t = ospool.tile([64, 256], f32, tag="os",
                                           name=f"os{b}_{t}")
                        nc.scalar.activation(
                            os_t[:], po_t[:],
                            mybir.ActivationFunctionType.Identity,
                            bias=bias_t[:, :1])
                        nc.sync.dma_start(
                            out_d.ap()[b:b + 1, :, r0:r0 + ncols]
                            .rearrange("b o r -> (b o) r"),
                            os_t[:, :ncols])

    nc.compile()
    return nc


def _prep(inputs):
    """Sort edges by (tile, col-half, col); compute global chunk counts and
    fill flat slot arrays (idx, one-hot / rloc+val)."""
    ops = []
    for name in ("L", "EW", "NS"):
        row = np.asarray(inputs[f"{name}_row"]).astype(np.int64)
        col = np.asarray(inputs[f"{name}_col"]).astype(np.int64)
        val = np.asarray(inputs[f"{name}_val"]).astype(np.float32)
        t = row >> 7
        h = (col >= SPLIT).astype(np.int64)
        order = np.lexsort((col, h, t))
        row, col, val, t, h = (a[order] for a in (row, col, val, t, h))
        grp = t * 2 + h
        counts = np.bincount(grp, minlength=NTILE * 2)
        starts = np.zeros(NTILE * 2, np.int64)
        starts[1:] = np.cumsum(counts)[:-1]
        pos = np.arange(len(row)) - starts[grp]
        C0 = int(np.ceil(max(int(counts[0::2].max()), 1) / 128))
        C1 = int(np.ceil(max(int(counts[1::2].max()), 1) / 128))
        ops.append((row, col, val, t, h, pos, C0, C1))

    C0s = [o[6] for o in ops]
    C1s = [o[7] for o in ops]
    CLO = sum(C0s)
    STOT = CLO + sum(C1s)
    lo_base = [0, C0s[0], C0s[0] + C0s[1]]
    hi_base = [CLO, CLO + C1s[0], CLO + C1s[0] + C1s[1]]

    idxf = np.full(NTILE * STOT * 128, -1, np.int16)
    ohf = np.zeros((NTILE * STOT * 128, 128), np.float16)
    rlocf = np.zeros(NTILE * STOT * 128, np.float32)
    valf = np.zeros(NTILE * STOT * 128, np.float32)
    gcnt = np.zeros((NTILE, 6), np.int64)  # per (tile, group) edge counts
    for i, (row, col, val, t, h, pos, _, _) in enumerate(ops):
        cb = np.where(h == 0, lo_base[i], hi_base[i])
        slot = (t * STOT + cb) * 128 + pos
        idxf[slot] = (col - h * SPLIT).astype(np.int16)
        ohf[slot, row & 127] = val.astype(np.float16)
        rlocf[slot] = (row & 127).astype(np.float32)
        valf[slot] = val.astype(np.float32)
        grp = i + np.where(h == 0, 0, 3)
        np.add.at(gcnt, (t, grp), 1)

    S = tuple(zip(C0s, C1s))
    return S, STOT, idxf, ohf, rlocf, valf, gcnt


def kernel(**inputs):
    x = np.asarray(inputs["x"], dtype=np.float32)
    coeffs = np.asarray(inputs["coeffs"], dtype=np.float32)
    bias = np.asarray(inputs["bias"], dtype=np.float32)

    S, STOT, idxf, ohf, rlocf, valf, gcnt = _prep(inputs)

    key = (S, OH_MODE)
    if key not in _cache:
        _cache[key] = _build(S, OH_MODE)
    nc = _cache[key]

    xT = np.zeros((NVPAD, BC), np.float16)
    xT[:NV] = x.transpose(2, 0, 1).reshape(NV, BC)
    xf = np.zeros((BC, NVPAD), np.float16)
    xf[:, :NV] = x.reshape(BC, NV)
    coef16 = np.tile(
        coeffs.transpose(1, 0, 2).reshape(64, 256).astype(np.float16), (2, 1))
    bias2 = bias.reshape(64, 1).astype(np.float32)
    iota16 = np.broadcast_to(
        np.arange(128, dtype=np.float16), (128, 128)).copy()

    idxT = idxf.reshape(NTILE, STOT * 128)
    ohT = ohf.reshape(NTILE, STOT, 128, 128)
    rlocT = rlocf.reshape(NTILE, STOT, 128)
    valT = valf.reshape(NTILE, STOT, 128)
    DCH = [cj for cj in range(STOT) if cj % DVE_EVERY == 0]
    SCH = [cj for cj in range(STOT) if cj % DVE_EVERY != 0]
    pieces = _pieces(S)
    NP = len(pieces)
    # per (tile, piece) valid counts, trailing-trimmed, floored at 16
    cnts = np.zeros((NTILE, NP), np.int32)
    grp_of_piece = []
    C0s = [c0 for c0, _ in S]
    C1s = [c1 for _, c1 in S]
    gbases = np.cumsum([0] + [C0s[0], C0s[1], C0s[2], C1s[0], C1s[1]])
    for pi, (coff, nch, hi, gbase) in enumerate(pieces):
        gi = int(np.searchsorted(gbases, gbase, side="right") - 1)
        start = (coff - gbase) * 128
        c = np.clip(gcnt[:, gi] - start, 0, nch * 128)
        if TRIM == "regfull":
            cnts[:, pi] = nch * 128
            continue
        cnts[:, pi] = np.maximum(c, 16)
        # ensure the >=16 floor has non-negative idxs to match the count
        for t in np.nonzero(c < 16)[0]:
            s0 = (t * STOT + coff) * 128 + int(c[t])
            need = 16 - int(c[t])
            idxf[s0:s0 + need] = np.maximum(idxf[s0:s0 + need], 0)
    if TRIM == "on":
        # first 2 tiles of each core gather everything: pad idx 0
        for core in range(NCORES):
            for t in (core * TPC, core * TPC + 1):
                a, b = t * STOT * 128, (t + 1) * STOT * 128
                np.maximum(idxf[a:b], 0, out=idxf[a:b])
    else:
        np.maximum(idxf, 0, out=idxf)

    in_maps = []
    for core in range(NCORES):
        t0, t1 = core * TPC, (core + 1) * TPC
        n = STOT * 128
        A = idxT[t0:t1]
        W = A.reshape(TPC, n // 16, 16).transpose(0, 2, 1)
        W = np.tile(W, (1, 8, 1))
        idx16 = np.ascontiguousarray(
            W.transpose(1, 0, 2).reshape(128, TPC * n // 16))
        xi = np.concatenate(
            [xf[:128, t0 * 128:t1 * 128], xf[128:, t0 * 128:t1 * 128]],
            axis=1)
        m = {
            "xg": xT,
            "xI": np.ascontiguousarray(xi),
            "idx16": idx16,
            "coef": coef16,
            "bias2": bias2,
            "iota16": iota16,
            "cnt": np.ascontiguousarray(
                cnts[t0:t1].reshape(1, TPC * NP)),
            "oh": np.ascontiguousarray(
                ohT[t0:t1][:, SCH].transpose(2, 0, 1, 3).reshape(128, -1)),
            "rloc": np.ascontiguousarray(
                rlocT[t0:t1][:, DCH].transpose(2, 0, 1).reshape(128, -1)),
            "val": np.ascontiguousarray(
                valT[t0:t1][:, DCH].transpose(2, 0, 1).reshape(128, -1)),
        }
        in_maps.append(m)

    res = run_bass_kernel_spmd(nc, in_maps, core_ids=list(range(NCORES)))
    out = np.concatenate([res.results[c]["out"] for c in range(NCORES)],
                         axis=2)
    return np.ascontiguousarray(out[:, :, :NV])


# revision 18
# speedup vs baseline: 1.7861x; 1.0291x over previous
"""MeshConv (gnn_message_passing) Bass kernel for 8 trn2 NeuronCores.

out[b,o,v] = bias[o] + sum_k coeffs[k,:,o]^T feats_k[b,v,:]
  feats_0 = x^T (identity), feats_{1,2,3} = spmm(L/EW/NS, x)

Strategy: shard output vertices across cores (row-partitioned spmm),
fp16 data path.  Edges sorted by (dest tile, col-half, col) into 128-edge
chunks; per tile one dma_gather per <=8-chunk range pulls 512B fp16 rows
of x^T (int16 indices split lo/hi around row 32768).  One-hot matrices
[edge, row_local]*val are precomputed on the host in fp16 and streamed
from HBM (no DVE work).  Per chunk the PE computes
yT[bc_half, row] += g[edge, bc_half]^T @ oh[edge, row] directly in the
transposed layout, so no PE transposes are needed; identity features are
DMA'd straight from x (natural [bc, v] layout).  Per tile pair the
per-operator coeffs hit yT (free dim 256), bias added on the Scalar
engine, output written as [o, rows] slabs per batch.
"""

import sys

sys.path.insert(0, "/opt/trn_rl_repo")

import numpy as np

import concourse.bass as bass
import concourse.bacc as bacc
import concourse.tile as tile
import concourse.mybir as mybir
from concourse.bass_utils import run_bass_kernel_spmd

NV = 40962
B = 4
C = 64
BC = B * C  # 256
NCORES = 8
NTILE = 328          # 128-row tiles, 328*128 = 41984 >= 40962
NVPAD = NTILE * 128
TPC = NTILE // NCORES  # 41 tiles per core
TPCR = TPC * 128
SPLIT = 32768        # int16 index split point
MAXD = 4             # max chunks per gather call (512 descs: 2 fit the ring)
NQ = 4               # SWDGE queues (ucode max)

OH_MODE = "hybrid"   # stream most one-hots from HBM, build every DVE_EVERY-th on DVE
DVE_EVERY = 0        # 0: stream all one-hots; else build every Nth on DVE
TRIM = "on"          # "on": runtime-trimmed counts; "regfull": registers but
                     # full counts (debug); "off": static num_idxs

_cache = {}

OPK = ("L", "E", "N")


def _pieces(S):
    """Group-aligned gather pieces: (chunk_off, nchunks, is_hi, grp_start).

    Each piece stays inside one (op, half) group so slot padding is always
    trailing within the piece and can be trimmed via num_idxs_reg."""
    C0s = [c0 for c0, _ in S]
    C1s = [c1 for _, c1 in S]
    groups = [(C0s[0], 0), (C0s[1], 0), (C0s[2], 0),
              (C1s[0], 1), (C1s[1], 1), (C1s[2], 1)]
    pieces = []
    base = 0
    for Cg, hi in groups:
        a = 0
        while a < Cg:
            n = min(MAXD, Cg - a)
            pieces.append((base + a, n, hi, base))
            a += n
        base += Cg
    return pieces


def _build(S, oh_mode):
    """Build the per-core Bass program for ((CL0,CL1),(CE0,CE1),(CN0,CN1))."""
    C0s = [c0 for c0, _ in S]
    C1s = [c1 for _, c1 in S]
    CLO, CHI = sum(C0s), sum(C1s)
    STOT = CLO + CHI
    lo_base = [0, C0s[0], C0s[0] + C0s[1]]
    hi_base = [CLO, CLO + C1s[0], CLO + C1s[0] + C1s[1]]
    op_chunks = {
        k: list(range(lo_base[i], lo_base[i] + C0s[i]))
        + list(range(hi_base[i], hi_base[i] + C1s[i]))
        for i, k in enumerate(OPK)
    }
    f32 = mybir.dt.float32
    f16 = mybir.dt.float16

    nc = bacc.Bacc("TRN2", target_bir_lowering=False, debug=False,
                   num_devices=NCORES, num_swdge_queues=NQ)

    xg_d = nc.dram_tensor("xg", [NVPAD, BC], f16, kind="ExternalInput")
    xI_d = nc.dram_tensor("xI", [128, 2 * TPCR], f16, kind="ExternalInput")
    idx_d = nc.dram_tensor("idx16", [128, TPC * STOT * 8], mybir.dt.int16,
                           kind="ExternalInput")
    DCH = [cj for cj in range(STOT)
           if DVE_EVERY and cj % DVE_EVERY == 0]
    SCH = [cj for cj in range(STOT) if cj not in DCH]
    ND, NS = len(DCH), len(SCH)
    chpos = {}
    for i, cj in enumerate(DCH):
        chpos[cj] = ("v", i)
    for i, cj in enumerate(SCH):
        chpos[cj] = ("s", i)
    oh_d = nc.dram_tensor("oh", [128, TPC * NS * 128], f16,
                          kind="ExternalInput")
    if ND:
        iota_d = nc.dram_tensor("iota16", [128, 128], f16,
                                kind="ExternalInput")
        rloc_d = nc.dram_tensor("rloc", [128, TPC * ND], f32,
                                kind="ExternalInput")
        val_d = nc.dram_tensor("val", [128, TPC * ND], f32,
                               kind="ExternalInput")
    cnt_d = nc.dram_tensor("cnt", [1, TPC * len(_pieces(S))], mybir.dt.int32,
                           kind="ExternalInput")
    coef_d = nc.dram_tensor("coef", [128, 256], f16, kind="ExternalInput")
    bias_d = nc.dram_tensor("bias2", [64, 1], f32, kind="ExternalInput")
    out_d = nc.dram_tensor("out", [B, C, TPC * 128], f32,
                           kind="ExternalOutput")

    pieces = _pieces(S)
    NP = len(pieces)

    with tile.TileContext(nc) as tc:
        with (
            tc.tile_pool(name="const", bufs=1) as cpool,
            tc.tile_pool(name="meta", bufs=1) as mpool,
            tc.tile_pool(name="g", bufs=2) as gpool,
            tc.tile_pool(name="oh", bufs=2) as ohpool,
            tc.tile_pool(name="yt", bufs=2) as ytpool,
            tc.tile_pool(name="os", bufs=2) as ospool,
            tc.tile_pool(name="py", bufs=1, space="PSUM") as pypool,
            tc.tile_pool(name="po", bufs=2, space="PSUM") as popool,
        ):
            # constants
            coef_t = cpool.tile([128, 256], f16)
            nc.sync.dma_start(coef_t[:], coef_d.ap()[:])
            bias_t = cpool.tile([64, 1], f32)
            nc.sync.dma_start(bias_t[:], bias_d.ap()[:])
            idx_t = mpool.tile([128, TPC * STOT * 8], mybir.dt.int16)
            nc.sync.dma_start(idx_t[:], idx_d.ap()[:])
            cnt_t = mpool.tile([1, TPC * NP], mybir.dt.int32)
            nc.sync.dma_start(cnt_t[:], cnt_d.ap()[:])
            if ND:
                iota_t = cpool.tile([128, 128], f16)
                nc.sync.dma_start(iota_t[:], iota_d.ap()[:])
                rloc_t = mpool.tile([128, TPC * ND], f32)
                nc.sync.dma_start(rloc_t[:], rloc_d.ap()[:])
                val_t = mpool.tile([128, TPC * ND], f32)
                nc.sync.dma_start(val_t[:], val_d.ap()[:])

            yts = {}  # (k, hb) -> staging tile [128, 256] f16 per tile pair
            qn = 0
            creg = nc.gpsimd.alloc_register("cnt_reg")

            for t in range(TPC):
                pair_off = (t % 2) * 128
                is_pair_start = t % 2 == 0
                is_orphan = t == TPC - 1 and is_pair_start

                g_t = gpool.tile([128, STOT * BC], f16, tag="g")
                ib = t * STOT * 8
                for pi, (coff, nch, hi, _) in enumerate(pieces):
                    src = xg_d.ap()[SPLIT:, :] if hi else xg_d.ap()[:SPLIT, :]
                    if t < 2 or TRIM == "off":
                        nreg = nch * 128
                    else:
                        ci = t * NP + pi
                        nc.gpsimd.reg_load(creg, cnt_t[0:1, ci:ci + 1])
                        nreg = creg
                    nc.gpsimd.dma_gather(
                        out_ap=g_t[:, coff * BC:(coff + nch) * BC]
                        .rearrange("p (j f) -> p j f", f=BC),
                        in_ap=src,
                        idxs_ap=idx_t[:, ib + coff * 8:ib + (coff + nch) * 8],
                        num_idxs=nch * 128,
                        num_idxs_reg=nreg,
                        elem_size=BC,
                        queue_num=qn % NQ,
                    )
                    qn += 1

                oh_s = ohpool.tile([128, NS * 128], f16, tag="ohs")
                nc.sync.dma_start(
                    oh_s[:], oh_d.ap()[:, t * NS * 128:(t + 1) * NS * 128])
                oh_v = (ohpool.tile([128, ND * 128], f16, tag="ohv")
                        if ND else None)
                for i in range(ND):
                    mcol = t * ND + i
                    nc.vector.tensor_scalar(
                        out=oh_v[:, i * 128:(i + 1) * 128],
                        in0=iota_t[:],
                        scalar1=rloc_t[:, mcol:mcol + 1],
                        scalar2=val_t[:, mcol:mcol + 1],
                        op0=mybir.AluOpType.is_equal,
                        op1=mybir.AluOpType.mult,
                    )

                def oh_ap(cj):
                    kind, i = chpos[cj]
                    tl = oh_v if kind == "v" else oh_s
                    return tl[:, i * 128:(i + 1) * 128]

                if is_pair_start:
                    for k in ("I",) + OPK:
                        for hb in (0, 1):
                            yts[(k, hb)] = ytpool.tile(
                                [128, 256], f16, tag=f"yt{k}{hb}",
                                name=f"yt{k}{hb}_{t}")
                            if is_orphan:
                                nc.vector.memset(
                                    yts[(k, hb)][:].bitcast(f32), 0.0)

                # identity features straight from x (natural layout)
                for hb in (0, 1):
                    nc.sync.dma_start(
                        yts[("I", hb)][:, pair_off:pair_off + 128],
                        xI_d.ap()[:, hb * TPCR + t * 128:
                                  hb * TPCR + (t + 1) * 128])

                # spmm: yT[bc_half, row] += g[e, bc_half]^T @ oh[e, row]
                for k in OPK:
                    chunks = op_chunks[k]
                    for hb in (0, 1):
                        py_t = pypool.tile([128, 128], f32, tag=f"py{k}{hb}")
                        for ci, cj in enumerate(chunks):
                            nc.tensor.matmul(
                                py_t[:],
                                g_t[:, cj * BC + hb * 128:
                                    cj * BC + hb * 128 + 128],
                                oh_ap(cj),
                                start=(ci == 0),
                                stop=(ci == len(chunks) - 1),
                            )
                        nc.scalar.activation(
                            yts[(k, hb)][:, pair_off:pair_off + 128], py_t[:],
                            mybir.ActivationFunctionType.Copy)

                # coeffs matmuls on completed pair
                if not is_pair_start or is_orphan:
                    r0 = (t - 1 if not is_pair_start else t) * 128
                    ncols = 128 if is_orphan else 256
                    for b in range(B):
                        po_t = popool.tile([64, 256], f32, tag="po",
                                           name=f"po{b}_{t}")
                        for ki, k in enumerate(("I",) + OPK):
                            p0 = (b % 2) * 64
                            rhs = yts[(k, b // 2)][p0:p0 + 64, :]
                            nc.tensor.matmul(
                                po_t[:],
                                coef_t[p0:p0 + 64, ki * 64:(ki + 1) * 64],
                                rhs,
                                start=(ki == 0),
                                stop=(ki == 3),
                            )
                        os_t = ospool.tile([64, 256], f32, tag="os",
                                           name=f"os{b}_{t}")
                        nc.scalar.activation(
                            os_t[:], po_t[:],
                            mybir.ActivationFunctionType.Identity,
                            bias=bias_t[:, :1])
                        nc.sync.dma_start(
                            out_d.ap()[b:b + 1, :, r0:r0 + ncols]
                            .rearrange("b o r -> (b o) r"),
                            os_t[:, :ncols])

    nc.compile()
    return nc


def _prep(inputs):
    """Sort edges by (tile, col-half, col); compute global chunk counts and
    fill flat slot arrays (idx, one-hot / rloc+val)."""
    ops = []
    for name in ("L", "EW", "NS"):
        row = np.asarray(inputs[f"{name}_row"]).astype(np.int64)
        col = np.asarray(inputs[f"{name}_col"]).astype(np.int64)
        val = np.asarray(inputs[f"{name}_val"]).astype(np.float32)
        t = row >> 7
        h = (col >= SPLIT).astype(np.int64)
        order = np.lexsort((col, h, t))
        row, col, val, t, h = (a[order] for a in (row, col, val, t, h))
        grp = t * 2 + h
        counts = np.bincount(grp, minlength=NTILE * 2)
        starts = np.zeros(NTILE * 2, np.int64)
        starts[1:] = np.cumsum(counts)[:-1]
        pos = np.arange(len(row)) - starts[grp]
        C0 = int(np.ceil(max(int(counts[0::2].max()), 1) / 128))
        C1 = int(np.ceil(max(int(counts[1::2].max()), 1) / 128))
        ops.append((row, col, val, t, h, pos, C0, C1))

    C0s = [o[6] for o in ops]
    C1s = [o[7] for o in ops]
    CLO = sum(C0s)
    STOT = CLO + sum(C1s)
    lo_base = [0, C0s[0], C0s[0] + C0s[1]]
    hi_base = [CLO, CLO + C1s[0], CLO + C1s[0] + C1s[1]]

    idxf = np.full(NTILE * STOT * 128, -1, np.int16)
    ohf = np.zeros((NTILE * STOT * 128, 128), np.float16)
    rlocf = np.zeros(NTILE * STOT * 128, np.float32)
    valf = np.zeros(NTILE * STOT * 128, np.float32)
    gcnt = np.zeros((NTILE, 6), np.int64)  # per (tile, group) edge counts
    for i, (row, col, val, t, h, pos, _, _) in enumerate(ops):
        cb = np.where(h == 0, lo_base[i], hi_base[i])
        slot = (t * STOT + cb) * 128 + pos
        idxf[slot] = (col - h * SPLIT).astype(np.int16)
        ohf[slot, row & 127] = val.astype(np.float16)
        rlocf[slot] = (row & 127).astype(np.float32)
        valf[slot] = val.astype(np.float32)
        grp = i + np.where(h == 0, 0, 3)
        np.add.at(gcnt, (t, grp), 1)

    S = tuple(zip(C0s, C1s))
    return S, STOT, idxf, ohf, rlocf, valf, gcnt


def kernel(**inputs):
    x = np.asarray(inputs["x"], dtype=np.float32)
    coeffs = np.asarray(inputs["coeffs"], dtype=np.float32)
    bias = np.asarray(inputs["bias"], dtype=np.float32)

    S, STOT, idxf, ohf, rlocf, valf, gcnt = _prep(inputs)

    key = (S, OH_MODE)
    if key not in _cache:
        _cache[key] = _build(S, OH_MODE)
    nc = _cache[key]

    xT = np.zeros((NVPAD, BC), np.float16)
    xT[:NV] = x.transpose(2, 0, 1).reshape(NV, BC)
    xf = np.zeros((BC, NVPAD), np.float16)
    xf[:, :NV] = x.reshape(BC, NV)
    coef16 = np.tile(
        coeffs.transpose(1, 0, 2).reshape(64, 256).astype(np.float16), (2, 1))
    bias2 = bias.reshape(64, 1).astype(np.float32)
    iota16 = np.broadcast_to(
        np.arange(128, dtype=np.float16), (128, 128)).copy()

    idxT = idxf.reshape(NTILE, STOT * 128)
    ohT = ohf.reshape(NTILE, STOT, 128, 128)
    rlocT = rlocf.reshape(NTILE, STOT, 128)
    valT = valf.reshape(NTILE, STOT, 128)
    DCH = [cj for cj in range(STOT)
           if DVE_EVERY and cj % DVE_EVERY == 0]
    SCH = [cj for cj in range(STOT) if cj not in DCH]
    pieces = _pieces(S)
    NP = len(pieces)
    # per (tile, piece) valid counts, trailing-trimmed, floored at 16
    cnts = np.zeros((NTILE, NP), np.int32)
    grp_of_piece = []
    C0s = [c0 for c0, _ in S]
    C1s = [c1 for _, c1 in S]
    gbases = np.cumsum([0] + [C0s[0], C0s[1], C0s[2], C1s[0], C1s[1]])
    for pi, (coff, nch, hi, gbase) in enumerate(pieces):
        gi = int(np.searchsorted(gbases, gbase, side="right") - 1)
        start = (coff - gbase) * 128
        c = np.clip(gcnt[:, gi] - start, 0, nch * 128)
        if TRIM == "regfull":
            cnts[:, pi] = nch * 128
            continue
        cnts[:, pi] = np.maximum(c, 16)
        # ensure the >=16 floor has non-negative idxs to match the count
        for t in np.nonzero(c < 16)[0]:
            s0 = (t * STOT + coff) * 128 + int(c[t])
            need = 16 - int(c[t])
            idxf[s0:s0 + need] = np.maximum(idxf[s0:s0 + need], 0)
    if TRIM == "on":
        # first 2 tiles of each core gather everything: pad idx 0
        for core in range(NCORES):
            for t in (core * TPC, core * TPC + 1):
                a, b = t * STOT * 128, (t + 1) * STOT * 128
                np.maximum(idxf[a:b], 0, out=idxf[a:b])
    else:
        np.maximum(idxf, 0, out=idxf)

    in_maps = []
    for core in range(NCORES):
        t0, t1 = core * TPC, (core + 1) * TPC
        n = STOT * 128
        A = idxT[t0:t1]
        W = A.reshape(TPC, n // 16, 16).transpose(0, 2, 1)
        W = np.tile(W, (1, 8, 1))
        idx16 = np.ascontiguousarray(
            W.transpose(1, 0, 2).reshape(128, TPC * n // 16))
        xi = np.concatenate(
            [xf[:128, t0 * 128:t1 * 128], xf[128:, t0 * 128:t1 * 128]],
            axis=1)
        m = {
            "xg": xT,
            "xI": np.ascontiguousarray(xi),
            "idx16": idx16,
            "coef": coef16,
            "bias2": bias2,
            "cnt": np.ascontiguousarray(
                cnts[t0:t1].reshape(1, TPC * NP)),
            "oh": np.ascontiguousarray(
                ohT[t0:t1][:, SCH].transpose(2, 0, 1, 3).reshape(128, -1)),
        }
        if DCH:
            m["iota16"] = iota16
            m["rloc"] = np.ascontiguousarray(
                rlocT[t0:t1][:, DCH].transpose(2, 0, 1).reshape(128, -1))
            m["val"] = np.ascontiguousarray(
                valT[t0:t1][:, DCH].transpose(2, 0, 1).reshape(128, -1))
        in_maps.append(m)

    res = run_bass_kernel_spmd(nc, in_maps, core_ids=list(range(NCORES)))
    out = np.concatenate([res.results[c]["out"] for c in range(NCORES)],
                         axis=2)
    return np.ascontiguousarray(out[:, :, :NV])


# revision 19
# speedup vs baseline: 1.7890x; 1.0016x over previous
"""MeshConv (gnn_message_passing) Bass kernel for 8 trn2 NeuronCores.

out[b,o,v] = bias[o] + sum_k coeffs[k,:,o]^T feats_k[b,v,:]
  feats_0 = x^T (identity), feats_{1,2,3} = spmm(L/EW/NS, x)

Strategy: shard output vertices across cores (row-partitioned spmm),
fp16 data path.  Edges sorted by (dest tile, col-half, col) into 128-edge
chunks; per tile one dma_gather per <=8-chunk range pulls 512B fp16 rows
of x^T (int16 indices split lo/hi around row 32768).  One-hot matrices
[edge, row_local]*val are precomputed on the host in fp16 and streamed
from HBM (no DVE work).  Per chunk the PE computes
yT[bc_half, row] += g[edge, bc_half]^T @ oh[edge, row] directly in the
transposed layout, so no PE transposes are needed; identity features are
DMA'd straight from x (natural [bc, v] layout).  Per tile pair the
per-operator coeffs hit yT (free dim 256), bias added on the Scalar
engine, output written as [o, rows] slabs per batch.
"""

import sys

sys.path.insert(0, "/opt/trn_rl_repo")

import numpy as np

import concourse.bass as bass
import concourse.bacc as bacc
import concourse.tile as tile
import concourse.mybir as mybir
from concourse.bass_utils import run_bass_kernel_spmd

NV = 40962
B = 4
C = 64
BC = B * C  # 256
NCORES = 8
NTILE = 328          # 128-row tiles, 328*128 = 41984 >= 40962
NVPAD = NTILE * 128
TPC = NTILE // NCORES  # 41 tiles per core
TPCR = TPC * 128
SPLIT = 32768        # int16 index split point
MAXD = 4             # max chunks per gather call (512 descs: 2 fit the ring)
NQ = 4               # SWDGE queues (ucode max)

OH_MODE = "hybrid"   # stream most one-hots from HBM, build every DVE_EVERY-th on DVE
DVE_EVERY = 0        # 0: stream all one-hots; else build every Nth on DVE
TRIM = "on"          # "on": runtime-trimmed counts; "regfull": registers but
                     # full counts (debug); "off": static num_idxs

_cache = {}

OPK = ("L", "E", "N")


def _pieces(S):
    """Group-aligned gather pieces: (chunk_off, nchunks, is_hi, grp_start).

    Each piece stays inside one (op, half) group so slot padding is always
    trailing within the piece and can be trimmed via num_idxs_reg."""
    C0s = [c0 for c0, _ in S]
    C1s = [c1 for _, c1 in S]
    groups = [(C0s[0], 0), (C0s[1], 0), (C0s[2], 0),
              (C1s[0], 1), (C1s[1], 1), (C1s[2], 1)]
    pieces = []
    base = 0
    for Cg, hi in groups:
        a = 0
        while a < Cg:
            n = min(MAXD, Cg - a)
            pieces.append((base + a, n, hi, base))
            a += n
        base += Cg
    return pieces


def _build(S, oh_mode):
    """Build the per-core Bass program for ((CL0,CL1),(CE0,CE1),(CN0,CN1))."""
    C0s = [c0 for c0, _ in S]
    C1s = [c1 for _, c1 in S]
    CLO, CHI = sum(C0s), sum(C1s)
    STOT = CLO + CHI
    lo_base = [0, C0s[0], C0s[0] + C0s[1]]
    hi_base = [CLO, CLO + C1s[0], CLO + C1s[0] + C1s[1]]
    op_chunks = {
        k: list(range(lo_base[i], lo_base[i] + C0s[i]))
        + list(range(hi_base[i], hi_base[i] + C1s[i]))
        for i, k in enumerate(OPK)
    }
    f32 = mybir.dt.float32
    f16 = mybir.dt.float16

    nc = bacc.Bacc("TRN2", target_bir_lowering=False, debug=False,
                   num_devices=NCORES, num_swdge_queues=NQ)

    xg_d = nc.dram_tensor("xg", [NVPAD, BC], f16, kind="ExternalInput")
    xI_d = nc.dram_tensor("xI", [128, 2 * TPCR], f16, kind="ExternalInput")
    idx_d = nc.dram_tensor("idx16", [128, TPC * STOT * 8], mybir.dt.int16,
                           kind="ExternalInput")
    DCH = [cj for cj in range(STOT)
           if DVE_EVERY and cj % DVE_EVERY == 0]
    SCH = [cj for cj in range(STOT) if cj not in DCH]
    ND, NS = len(DCH), len(SCH)
    chpos = {}
    for i, cj in enumerate(DCH):
        chpos[cj] = ("v", i)
    for i, cj in enumerate(SCH):
        chpos[cj] = ("s", i)
    oh_d = nc.dram_tensor("oh", [128, TPC * NS * 128], f16,
                          kind="ExternalInput")
    if ND:
        iota_d = nc.dram_tensor("iota16", [128, 128], f16,
                                kind="ExternalInput")
        rloc_d = nc.dram_tensor("rloc", [128, TPC * ND], f32,
                                kind="ExternalInput")
        val_d = nc.dram_tensor("val", [128, TPC * ND], f32,
                               kind="ExternalInput")
    cnt_d = nc.dram_tensor("cnt", [1, TPC * len(_pieces(S))], mybir.dt.int32,
                           kind="ExternalInput")
    coef_d = nc.dram_tensor("coef", [128, 256], f16, kind="ExternalInput")
    bias_d = nc.dram_tensor("bias2", [64, 1], f32, kind="ExternalInput")
    out_d = nc.dram_tensor("out", [B, C, TPC * 128], f32,
                           kind="ExternalOutput")

    pieces = _pieces(S)
    NP = len(pieces)

    with tile.TileContext(nc) as tc:
        with (
            tc.tile_pool(name="const", bufs=1) as cpool,
            tc.tile_pool(name="meta", bufs=1) as mpool,
            tc.tile_pool(name="g", bufs=3) as gpool,
            tc.tile_pool(name="oh", bufs=3) as ohpool,
            tc.tile_pool(name="yt", bufs=2) as ytpool,
            tc.tile_pool(name="os", bufs=2) as ospool,
            tc.tile_pool(name="py", bufs=1, space="PSUM") as pypool,
            tc.tile_pool(name="po", bufs=2, space="PSUM") as popool,
        ):
            # constants
            coef_t = cpool.tile([128, 256], f16)
            nc.sync.dma_start(coef_t[:], coef_d.ap()[:])
            bias_t = cpool.tile([64, 1], f32)
            nc.sync.dma_start(bias_t[:], bias_d.ap()[:])
            idx_t = mpool.tile([128, TPC * STOT * 8], mybir.dt.int16)
            nc.sync.dma_start(idx_t[:], idx_d.ap()[:])
            cnt_t = mpool.tile([1, TPC * NP], mybir.dt.int32)
            nc.sync.dma_start(cnt_t[:], cnt_d.ap()[:])
            if ND:
                iota_t = cpool.tile([128, 128], f16)
                nc.sync.dma_start(iota_t[:], iota_d.ap()[:])
                rloc_t = mpool.tile([128, TPC * ND], f32)
                nc.sync.dma_start(rloc_t[:], rloc_d.ap()[:])
                val_t = mpool.tile([128, TPC * ND], f32)
                nc.sync.dma_start(val_t[:], val_d.ap()[:])

            yts = {}  # (k, hb) -> staging tile [128, 256] f16 per tile pair
            qn = 0
            creg = nc.gpsimd.alloc_register("cnt_reg")

            for t in range(TPC):
                pair_off = (t % 2) * 128
                is_pair_start = t % 2 == 0
                is_orphan = t == TPC - 1 and is_pair_start

                g_t = gpool.tile([128, STOT * BC], f16, tag="g")
                ib = t * STOT * 8
                for pi, (coff, nch, hi, _) in enumerate(pieces):
                    src = xg_d.ap()[SPLIT:, :] if hi else xg_d.ap()[:SPLIT, :]
                    if t < 2 or TRIM == "off":
                        nreg = nch * 128
                    else:
                        ci = t * NP + pi
                        nc.gpsimd.reg_load(creg, cnt_t[0:1, ci:ci + 1])
                        nreg = creg
                    nc.gpsimd.dma_gather(
                        out_ap=g_t[:, coff * BC:(coff + nch) * BC]
                        .rearrange("p (j f) -> p j f", f=BC),
                        in_ap=src,
                        idxs_ap=idx_t[:, ib + coff * 8:ib + (coff + nch) * 8],
                        num_idxs=nch * 128,
                        num_idxs_reg=nreg,
                        elem_size=BC,
                        queue_num=qn % NQ,
                    )
                    qn += 1

                oh_s = ohpool.tile([128, NS * 128], f16, tag="ohs")
                nc.sync.dma_start(
                    oh_s[:], oh_d.ap()[:, t * NS * 128:(t + 1) * NS * 128])
                oh_v = (ohpool.tile([128, ND * 128], f16, tag="ohv")
                        if ND else None)
                for i in range(ND):
                    mcol = t * ND + i
                    nc.vector.tensor_scalar(
                        out=oh_v[:, i * 128:(i + 1) * 128],
                        in0=iota_t[:],
                        scalar1=rloc_t[:, mcol:mcol + 1],
                        scalar2=val_t[:, mcol:mcol + 1],
                        op0=mybir.AluOpType.is_equal,
                        op1=mybir.AluOpType.mult,
                    )

                def oh_ap(cj):
                    kind, i = chpos[cj]
                    tl = oh_v if kind == "v" else oh_s
                    return tl[:, i * 128:(i + 1) * 128]

                if is_pair_start:
                    for k in ("I",) + OPK:
                        for hb in (0, 1):
                            yts[(k, hb)] = ytpool.tile(
                                [128, 256], f16, tag=f"yt{k}{hb}",
                                name=f"yt{k}{hb}_{t}")
                            if is_orphan:
                                nc.vector.memset(
                                    yts[(k, hb)][:].bitcast(f32), 0.0)

                # identity features straight from x (natural layout)
                for hb in (0, 1):
                    nc.sync.dma_start(
                        yts[("I", hb)][:, pair_off:pair_off + 128],
                        xI_d.ap()[:, hb * TPCR + t * 128:
                                  hb * TPCR + (t + 1) * 128])

                # spmm: yT[bc_half, row] += g[e, bc_half]^T @ oh[e, row]
                for k in OPK:
                    chunks = op_chunks[k]
                    for hb in (0, 1):
                        py_t = pypool.tile([128, 128], f32, tag=f"py{k}{hb}")
                        for ci, cj in enumerate(chunks):
                            nc.tensor.matmul(
                                py_t[:],
                                g_t[:, cj * BC + hb * 128:
                                    cj * BC + hb * 128 + 128],
                                oh_ap(cj),
                                start=(ci == 0),
                                stop=(ci == len(chunks) - 1),
                            )
                        nc.scalar.activation(
                            yts[(k, hb)][:, pair_off:pair_off + 128], py_t[:],
                            mybir.ActivationFunctionType.Copy)

                # coeffs matmuls on completed pair
                if not is_pair_start or is_orphan:
                    r0 = (t - 1 if not is_pair_start else t) * 128
                    ncols = 128 if is_orphan else 256
                    for b in range(B):
                        po_t = popool.tile([64, 256], f32, tag="po",
                                           name=f"po{b}_{t}")
                        for ki, k in enumerate(("I",) + OPK):
                            p0 = (b % 2) * 64
                            rhs = yts[(k, b // 2)][p0:p0 + 64, :]
                            nc.tensor.matmul(
                                po_t[:],
                                coef_t[p0:p0 + 64, ki * 64:(ki + 1) * 64],
                                rhs,
                                start=(ki == 0),
                                stop=(ki == 3),
                            )
                        os_t = ospool.tile([64, 256], f32, tag="os",
                                           name=f"os{b}_{t}")
                        nc.scalar.activation(
                            os_t[:], po_t[:],
                            mybir.ActivationFunctionType.Identity,
                            bias=bias_t[:, :1])
                        nc.sync.dma_start(
                            out_d.ap()[b:b + 1, :, r0:r0 + ncols]
                            .rearrange("b o r -> (b o) r"),
                            os_t[:, :ncols])

    nc.compile()
    return nc


def _prep(inputs):
    """Sort edges by (tile, col-half, col); compute global chunk counts and
    fill flat slot arrays (idx, one-hot / rloc+val)."""
    ops = []
    for name in ("L", "EW", "NS"):
        row = np.asarray(inputs[f"{name}_row"]).astype(np.int64)
        col = np.asarray(inputs[f"{name}_col"]).astype(np.int64)
        val = np.asarray(inputs[f"{name}_val"]).astype(np.float32)
        t = row >> 7
        h = (col >= SPLIT).astype(np.int64)
        order = np.lexsort((col, h, t))
        row, col, val, t, h = (a[order] for a in (row, col, val, t, h))
        grp = t * 2 + h
        counts = np.bincount(grp, minlength=NTILE * 2)
        starts = np.zeros(NTILE * 2, np.int64)
        starts[1:] = np.cumsum(counts)[:-1]
        pos = np.arange(len(row)) - starts[grp]
        C0 = int(np.ceil(max(int(counts[0::2].max()), 1) / 128))
        C1 = int(np.ceil(max(int(counts[1::2].max()), 1) / 128))
        ops.append((row, col, val, t, h, pos, C0, C1))

    C0s = [o[6] for o in ops]
    C1s = [o[7] for o in ops]
    CLO = sum(C0s)
    STOT = CLO + sum(C1s)
    lo_base = [0, C0s[0], C0s[0] + C0s[1]]
    hi_base = [CLO, CLO + C1s[0], CLO + C1s[0] + C1s[1]]

    idxf = np.full(NTILE * STOT * 128, -1, np.int16)
    ohf = np.zeros((NTILE * STOT * 128, 128), np.float16)
    rlocf = np.zeros(NTILE * STOT * 128, np.float32)
    valf = np.zeros(NTILE * STOT * 128, np.float32)
    gcnt = np.zeros((NTILE, 6), np.int64)  # per (tile, group) edge counts
    for i, (row, col, val, t, h, pos, _, _) in enumerate(ops):
        cb = np.where(h == 0, lo_base[i], hi_base[i])
        slot = (t * STOT + cb) * 128 + pos
        idxf[slot] = (col - h * SPLIT).astype(np.int16)
        ohf[slot, row & 127] = val.astype(np.float16)
        rlocf[slot] = (row & 127).astype(np.float32)
        valf[slot] = val.astype(np.float32)
        grp = i + np.where(h == 0, 0, 3)
        np.add.at(gcnt, (t, grp), 1)

    S = tuple(zip(C0s, C1s))
    return S, STOT, idxf, ohf, rlocf, valf, gcnt


def kernel(**inputs):
    x = np.asarray(inputs["x"], dtype=np.float32)
    coeffs = np.asarray(inputs["coeffs"], dtype=np.float32)
    bias = np.asarray(inputs["bias"], dtype=np.float32)

    S, STOT, idxf, ohf, rlocf, valf, gcnt = _prep(inputs)

    key = (S, OH_MODE)
    if key not in _cache:
        _cache[key] = _build(S, OH_MODE)
    nc = _cache[key]

    xT = np.zeros((NVPAD, BC), np.float16)
    xT[:NV] = x.transpose(2, 0, 1).reshape(NV, BC)
    xf = np.zeros((BC, NVPAD), np.float16)
    xf[:, :NV] = x.reshape(BC, NV)
    coef16 = np.tile(
        coeffs.transpose(1, 0, 2).reshape(64, 256).astype(np.float16), (2, 1))
    bias2 = bias.reshape(64, 1).astype(np.float32)
    iota16 = np.broadcast_to(
        np.arange(128, dtype=np.float16), (128, 128)).copy()

    idxT = idxf.reshape(NTILE, STOT * 128)
    ohT = ohf.reshape(NTILE, STOT, 128, 128)
    rlocT = rlocf.reshape(NTILE, STOT, 128)
    valT = valf.reshape(NTILE, STOT, 128)
    DCH = [cj for cj in range(STOT)
           if DVE_EVERY and cj % DVE_EVERY == 0]
    SCH = [cj for cj in range(STOT) if cj not in DCH]
    pieces = _pieces(S)
    NP = len(pieces)
    # per (tile, piece) valid counts, trailing-trimmed, floored at 16
    cnts = np.zeros((NTILE, NP), np.int32)
    grp_of_piece = []
    C0s = [c0 for c0, _ in S]
    C1s = [c1 for _, c1 in S]
    gbases = np.cumsum([0] + [C0s[0], C0s[1], C0s[2], C1s[0], C1s[1]])
    for pi, (coff, nch, hi, gbase) in enumerate(pieces):
        gi = int(np.searchsorted(gbases, gbase, side="right") - 1)
        start = (coff - gbase) * 128
        c = np.clip(gcnt[:, gi] - start, 0, nch * 128)
        if TRIM == "regfull":
            cnts[:, pi] = nch * 128
            continue
        cnts[:, pi] = np.maximum(c, 16)
        # ensure the >=16 floor has non-negative idxs to match the count
        for t in np.nonzero(c < 16)[0]:
            s0 = (t * STOT + coff) * 128 + int(c[t])
            need = 16 - int(c[t])
            idxf[s0:s0 + need] = np.maximum(idxf[s0:s0 + need], 0)
    if TRIM == "on":
        # first 2 tiles of each core gather everything: pad idx 0
        for core in range(NCORES):
            for t in (core * TPC, core * TPC + 1):
                a, b = t * STOT * 128, (t + 1) * STOT * 128
                np.maximum(idxf[a:b], 0, out=idxf[a:b])
    else:
        np.maximum(idxf, 0, out=idxf)

    in_maps = []
    for core in range(NCORES):
        t0, t1 = core * TPC, (core + 1) * TPC
        n = STOT * 128
        A = idxT[t0:t1]
        W = A.reshape(TPC, n // 16, 16).transpose(0, 2, 1)
        W = np.tile(W, (1, 8, 1))
        idx16 = np.ascontiguousarray(
            W.transpose(1, 0, 2).reshape(128, TPC * n // 16))
        xi = np.concatenate(
            [xf[:128, t0 * 128:t1 * 128], xf[128:, t0 * 128:t1 * 128]],
            axis=1)
        m = {
            "xg": xT,
            "xI": np.ascontiguousarray(xi),
            "idx16": idx16,
            "coef": coef16,
            "bias2": bias2,
            "cnt": np.ascontiguousarray(
                cnts[t0:t1].reshape(1, TPC * NP)),
            "oh": np.ascontiguousarray(
                ohT[t0:t1][:, SCH].transpose(2, 0, 1, 3).reshape(128, -1)),
        }
        if DCH:
            m["iota16"] = iota16
            m["rloc"] = np.ascontiguousarray(
                rlocT[t0:t1][:, DCH].transpose(2, 0, 1).reshape(128, -1))
            m["val"] = np.ascontiguousarray(
                valT[t0:t1][:, DCH].transpose(2, 0, 1).reshape(128, -1))
        in_maps.append(m)

    res = run_bass_kernel_spmd(nc, in_maps, core_ids=list(range(NCORES)))
    out = np.concatenate([res.results[c]["out"] for c in range(NCORES)],
                         axis=2)
    return np.ascontiguousarray(out[:, :, :NV])


# revision 20
# speedup vs baseline: 1.7985x; 1.0053x over previous
"""MeshConv (gnn_message_passing) Bass kernel for 8 trn2 NeuronCores.

out[b,o,v] = bias[o] + sum_k coeffs[k,:,o]^T feats_k[b,v,:]
  feats_0 = x^T (identity), feats_{1,2,3} = spmm(L/EW/NS, x)

Strategy: shard output vertices across cores (row-partitioned spmm),
fp16 data path.  Edges sorted by (dest tile, col-half, col) into 128-edge
chunks; per tile one dma_gather per <=8-chunk range pulls 512B fp16 rows
of x^T (int16 indices split lo/hi around row 32768).  One-hot matrices
[edge, row_local]*val are precomputed on the host in fp16 and streamed
from HBM (no DVE work).  Per chunk the PE computes
yT[bc_half, row] += g[edge, bc_half]^T @ oh[edge, row] directly in the
transposed layout, so no PE transposes are needed; identity features are
DMA'd straight from x (natural [bc, v] layout).  Per tile pair the
per-operator coeffs hit yT (free dim 256), bias added on the Scalar
engine, output written as [o, rows] slabs per batch.
"""

import sys

sys.path.insert(0, "/opt/trn_rl_repo")

import numpy as np

import concourse.bass as bass
import concourse.bacc as bacc
import concourse.tile as tile
import concourse.mybir as mybir
from concourse.bass_utils import run_bass_kernel_spmd

NV = 40962
B = 4
C = 64
BC = B * C  # 256
NCORES = 8
NTILE = 328          # 128-row tiles, 328*128 = 41984 >= 40962
NVPAD = NTILE * 128
TPC = NTILE // NCORES  # 41 tiles per core
TPCR = TPC * 128
SPLIT = 32768        # int16 index split point
MAXD = 8             # max chunks per gather call (1024 descs: 2 fit the 2048 ring)
NQ = 4               # SWDGE queues (ucode max)

OH_MODE = "hybrid"   # stream most one-hots from HBM, build every DVE_EVERY-th on DVE
DVE_EVERY = 0        # 0: stream all one-hots; else build every Nth on DVE
TRIM = "on"          # "on": runtime-trimmed counts; "regfull": registers but
                     # full counts (debug); "off": static num_idxs

_cache = {}

OPK = ("L", "E", "N")


def _pieces(S):
    """Group-aligned gather pieces: (chunk_off, nchunks, is_hi, grp_start).

    Each piece stays inside one (op, half) group so slot padding is always
    trailing within the piece and can be trimmed via num_idxs_reg."""
    C0s = [c0 for c0, _ in S]
    C1s = [c1 for _, c1 in S]
    groups = [(C0s[0], 0), (C0s[1], 0), (C0s[2], 0),
              (C1s[0], 1), (C1s[1], 1), (C1s[2], 1)]
    pieces = []
    base = 0
    for Cg, hi in groups:
        a = 0
        while a < Cg:
            n = min(MAXD, Cg - a)
            pieces.append((base + a, n, hi, base))
            a += n
        base += Cg
    return pieces


def _build(S, oh_mode):
    """Build the per-core Bass program for ((CL0,CL1),(CE0,CE1),(CN0,CN1))."""
    C0s = [c0 for c0, _ in S]
    C1s = [c1 for _, c1 in S]
    CLO, CHI = sum(C0s), sum(C1s)
    STOT = CLO + CHI
    lo_base = [0, C0s[0], C0s[0] + C0s[1]]
    hi_base = [CLO, CLO + C1s[0], CLO + C1s[0] + C1s[1]]
    op_chunks = {
        k: list(range(lo_base[i], lo_base[i] + C0s[i]))
        + list(range(hi_base[i], hi_base[i] + C1s[i]))
        for i, k in enumerate(OPK)
    }
    f32 = mybir.dt.float32
    f16 = mybir.dt.float16

    nc = bacc.Bacc("TRN2", target_bir_lowering=False, debug=False,
                   num_devices=NCORES, num_swdge_queues=NQ,
                   dynamic_dma_scratch_size=32768)

    xg_d = nc.dram_tensor("xg", [NVPAD, BC], f16, kind="ExternalInput")
    xI_d = nc.dram_tensor("xI", [128, 2 * TPCR], f16, kind="ExternalInput")
    idx_d = nc.dram_tensor("idx16", [128, TPC * STOT * 8], mybir.dt.int16,
                           kind="ExternalInput")
    DCH = [cj for cj in range(STOT)
           if DVE_EVERY and cj % DVE_EVERY == 0]
    SCH = [cj for cj in range(STOT) if cj not in DCH]
    ND, NS = len(DCH), len(SCH)
    chpos = {}
    for i, cj in enumerate(DCH):
        chpos[cj] = ("v", i)
    for i, cj in enumerate(SCH):
        chpos[cj] = ("s", i)
    oh_d = nc.dram_tensor("oh", [128, TPC * NS * 128], f16,
                          kind="ExternalInput")
    if ND:
        iota_d = nc.dram_tensor("iota16", [128, 128], f16,
                                kind="ExternalInput")
        rloc_d = nc.dram_tensor("rloc", [128, TPC * ND], f32,
                                kind="ExternalInput")
        val_d = nc.dram_tensor("val", [128, TPC * ND], f32,
                               kind="ExternalInput")
    cnt_d = nc.dram_tensor("cnt", [1, TPC * len(_pieces(S))], mybir.dt.int32,
                           kind="ExternalInput")
    coef_d = nc.dram_tensor("coef", [128, 256], f16, kind="ExternalInput")
    bias_d = nc.dram_tensor("bias2", [64, 1], f32, kind="ExternalInput")
    out_d = nc.dram_tensor("out", [B, C, TPC * 128], f32,
                           kind="ExternalOutput")

    pieces = _pieces(S)
    NP = len(pieces)

    with tile.TileContext(nc) as tc:
        with (
            tc.tile_pool(name="const", bufs=1) as cpool,
            tc.tile_pool(name="meta", bufs=1) as mpool,
            tc.tile_pool(name="g", bufs=3) as gpool,
            tc.tile_pool(name="oh", bufs=3) as ohpool,
            tc.tile_pool(name="yt", bufs=2) as ytpool,
            tc.tile_pool(name="os", bufs=2) as ospool,
            tc.tile_pool(name="py", bufs=1, space="PSUM") as pypool,
            tc.tile_pool(name="po", bufs=2, space="PSUM") as popool,
        ):
            # constants
            coef_t = cpool.tile([128, 256], f16)
            nc.sync.dma_start(coef_t[:], coef_d.ap()[:])
            bias_t = cpool.tile([64, 1], f32)
            nc.sync.dma_start(bias_t[:], bias_d.ap()[:])
            idx_t = mpool.tile([128, TPC * STOT * 8], mybir.dt.int16)
            nc.sync.dma_start(idx_t[:], idx_d.ap()[:])
            cnt_t = mpool.tile([1, TPC * NP], mybir.dt.int32)
            nc.sync.dma_start(cnt_t[:], cnt_d.ap()[:])
            if ND:
                iota_t = cpool.tile([128, 128], f16)
                nc.sync.dma_start(iota_t[:], iota_d.ap()[:])
                rloc_t = mpool.tile([128, TPC * ND], f32)
                nc.sync.dma_start(rloc_t[:], rloc_d.ap()[:])
                val_t = mpool.tile([128, TPC * ND], f32)
                nc.sync.dma_start(val_t[:], val_d.ap()[:])

            yts = {}  # (k, hb) -> staging tile [128, 256] f16 per tile pair
            qn = 0
            creg = nc.gpsimd.alloc_register("cnt_reg")

            for t in range(TPC):
                pair_off = (t % 2) * 128
                is_pair_start = t % 2 == 0
                is_orphan = t == TPC - 1 and is_pair_start

                g_t = gpool.tile([128, STOT * BC], f16, tag="g")
                ib = t * STOT * 8
                for pi, (coff, nch, hi, _) in enumerate(pieces):
                    src = xg_d.ap()[SPLIT:, :] if hi else xg_d.ap()[:SPLIT, :]
                    if t < 2 or TRIM == "off":
                        nreg = nch * 128
                    else:
                        ci = t * NP + pi
                        nc.gpsimd.reg_load(creg, cnt_t[0:1, ci:ci + 1])
                        nreg = creg
                    nc.gpsimd.dma_gather(
                        out_ap=g_t[:, coff * BC:(coff + nch) * BC]
                        .rearrange("p (j f) -> p j f", f=BC),
                        in_ap=src,
                        idxs_ap=idx_t[:, ib + coff * 8:ib + (coff + nch) * 8],
                        num_idxs=nch * 128,
                        num_idxs_reg=nreg,
                        elem_size=BC,
                        queue_num=qn % NQ,
                    )
                    qn += 1

                oh_s = ohpool.tile([128, NS * 128], f16, tag="ohs")
                nc.sync.dma_start(
                    oh_s[:], oh_d.ap()[:, t * NS * 128:(t + 1) * NS * 128])
                oh_v = (ohpool.tile([128, ND * 128], f16, tag="ohv")
                        if ND else None)
                for i in range(ND):
                    mcol = t * ND + i
                    nc.vector.tensor_scalar(
                        out=oh_v[:, i * 128:(i + 1) * 128],
                        in0=iota_t[:],
                        scalar1=rloc_t[:, mcol:mcol + 1],
                        scalar2=val_t[:, mcol:mcol + 1],
                        op0=mybir.AluOpType.is_equal,
                        op1=mybir.AluOpType.mult,
                    )

                def oh_ap(cj):
                    kind, i = chpos[cj]
                    tl = oh_v if kind == "v" else oh_s
                    return tl[:, i * 128:(i + 1) * 128]

                if is_pair_start:
                    for k in ("I",) + OPK:
                        for hb in (0, 1):
                            yts[(k, hb)] = ytpool.tile(
                                [128, 256], f16, tag=f"yt{k}{hb}",
                                name=f"yt{k}{hb}_{t}")
                            if is_orphan:
                                nc.vector.memset(
                                    yts[(k, hb)][:].bitcast(f32), 0.0)

                # identity features straight from x (natural layout)
                for hb in (0, 1):
                    nc.sync.dma_start(
                        yts[("I", hb)][:, pair_off:pair_off + 128],
                        xI_d.ap()[:, hb * TPCR + t * 128:
                                  hb * TPCR + (t + 1) * 128])

                # spmm: yT[bc_half, row] += g[e, bc_half]^T @ oh[e, row]
                for k in OPK:
                    chunks = op_chunks[k]
                    for hb in (0, 1):
                        py_t = pypool.tile([128, 128], f32, tag=f"py{k}{hb}")
                        for ci, cj in enumerate(chunks):
                            nc.tensor.matmul(
                                py_t[:],
                                g_t[:, cj * BC + hb * 128:
                                    cj * BC + hb * 128 + 128],
                                oh_ap(cj),
                                start=(ci == 0),
                                stop=(ci == len(chunks) - 1),
                            )
                        nc.scalar.activation(
                            yts[(k, hb)][:, pair_off:pair_off + 128], py_t[:],
                            mybir.ActivationFunctionType.Copy)

                # coeffs matmuls on completed pair
                if not is_pair_start or is_orphan:
                    r0 = (t - 1 if not is_pair_start else t) * 128
                    ncols = 128 if is_orphan else 256
                    for b in range(B):
                        po_t = popool.tile([64, 256], f32, tag="po",
                                           name=f"po{b}_{t}")
                        for ki, k in enumerate(("I",) + OPK):
                            p0 = (b % 2) * 64
                            rhs = yts[(k, b // 2)][p0:p0 + 64, :]
                            nc.tensor.matmul(
                                po_t[:],
                                coef_t[p0:p0 + 64, ki * 64:(ki + 1) * 64],
                                rhs,
                                start=(ki == 0),
                                stop=(ki == 3),
                            )
                        os_t = ospool.tile([64, 256], f32, tag="os",
                                           name=f"os{b}_{t}")
                        nc.scalar.activation(
                            os_t[:], po_t[:],
                            mybir.ActivationFunctionType.Identity,
                            bias=bias_t[:, :1])
                        nc.sync.dma_start(
                            out_d.ap()[b:b + 1, :, r0:r0 + ncols]
                            .rearrange("b o r -> (b o) r"),
                            os_t[:, :ncols])

    nc.compile()
    return nc


def _prep(inputs):
    """Sort edges by (tile, col-half, col); compute global chunk counts and
    fill flat slot arrays (idx, one-hot / rloc+val)."""
    ops = []
    for name in ("L", "EW", "NS"):
        row = np.asarray(inputs[f"{name}_row"]).astype(np.int64)
        col = np.asarray(inputs[f"{name}_col"]).astype(np.int64)
        val = np.asarray(inputs[f"{name}_val"]).astype(np.float32)
        t = row >> 7
        h = (col >= SPLIT).astype(np.int64)
        order = np.lexsort((col, h, t))
        row, col, val, t, h = (a[order] for a in (row, col, val, t, h))
        grp = t * 2 + h
        counts = np.bincount(grp, minlength=NTILE * 2)
        starts = np.zeros(NTILE * 2, np.int64)
        starts[1:] = np.cumsum(counts)[:-1]
        pos = np.arange(len(row)) - starts[grp]
        C0 = int(np.ceil(max(int(counts[0::2].max()), 1) / 128))
        C1 = int(np.ceil(max(int(counts[1::2].max()), 1) / 128))
        ops.append((row, col, val, t, h, pos, C0, C1))

    C0s = [o[6] for o in ops]
    C1s = [o[7] for o in ops]
    CLO = sum(C0s)
    STOT = CLO + sum(C1s)
    lo_base = [0, C0s[0], C0s[0] + C0s[1]]
    hi_base = [CLO, CLO + C1s[0], CLO + C1s[0] + C1s[1]]

    idxf = np.full(NTILE * STOT * 128, -1, np.int16)
    ohf = np.zeros((NTILE * STOT * 128, 128), np.float16)
    rlocf = np.zeros(NTILE * STOT * 128, np.float32)
    valf = np.zeros(NTILE * STOT * 128, np.float32)
    gcnt = np.zeros((NTILE, 6), np.int64)  # per (tile, group) edge counts
    for i, (row, col, val, t, h, pos, _, _) in enumerate(ops):
        cb = np.where(h == 0, lo_base[i], hi_base[i])
        slot = (t * STOT + cb) * 128 + pos
        idxf[slot] = (col - h * SPLIT).astype(np.int16)
        ohf[slot, row & 127] = val.astype(np.float16)
        rlocf[slot] = (row & 127).astype(np.float32)
        valf[slot] = val.astype(np.float32)
        grp = i + np.where(h == 0, 0, 3)
        np.add.at(gcnt, (t, grp), 1)

    S = tuple(zip(C0s, C1s))
    return S, STOT, idxf, ohf, rlocf, valf, gcnt


def kernel(**inputs):
    x = np.asarray(inputs["x"], dtype=np.float32)
    coeffs = np.asarray(inputs["coeffs"], dtype=np.float32)
    bias = np.asarray(inputs["bias"], dtype=np.float32)

    S, STOT, idxf, ohf, rlocf, valf, gcnt = _prep(inputs)

    key = (S, OH_MODE)
    if key not in _cache:
        _cache[key] = _build(S, OH_MODE)
    nc = _cache[key]

    xT = np.zeros((NVPAD, BC), np.float16)
    xT[:NV] = x.transpose(2, 0, 1).reshape(NV, BC)
    xf = np.zeros((BC, NVPAD), np.float16)
    xf[:, :NV] = x.reshape(BC, NV)
    coef16 = np.tile(
        coeffs.transpose(1, 0, 2).reshape(64, 256).astype(np.float16), (2, 1))
    bias2 = bias.reshape(64, 1).astype(np.float32)
    iota16 = np.broadcast_to(
        np.arange(128, dtype=np.float16), (128, 128)).copy()

    idxT = idxf.reshape(NTILE, STOT * 128)
    ohT = ohf.reshape(NTILE, STOT, 128, 128)
    rlocT = rlocf.reshape(NTILE, STOT, 128)
    valT = valf.reshape(NTILE, STOT, 128)
    DCH = [cj for cj in range(STOT)
           if DVE_EVERY and cj % DVE_EVERY == 0]
    SCH = [cj for cj in range(STOT) if cj not in DCH]
    pieces = _pieces(S)
    NP = len(pieces)
    # per (tile, piece) valid counts, trailing-trimmed, floored at 16
    cnts = np.zeros((NTILE, NP), np.int32)
    grp_of_piece = []
    C0s = [c0 for c0, _ in S]
    C1s = [c1 for _, c1 in S]
    gbases = np.cumsum([0] + [C0s[0], C0s[1], C0s[2], C1s[0], C1s[1]])
    for pi, (coff, nch, hi, gbase) in enumerate(pieces):
        gi = int(np.searchsorted(gbases, gbase, side="right") - 1)
        start = (coff - gbase) * 128
        c = np.clip(gcnt[:, gi] - start, 0, nch * 128)
        if TRIM == "regfull":
            cnts[:, pi] = nch * 128
            continue
        cnts[:, pi] = np.maximum(c, 16)
        # ensure the >=16 floor has non-negative idxs to match the count
        for t in np.nonzero(c < 16)[0]:
            s0 = (t * STOT + coff) * 128 + int(c[t])
            need = 16 - int(c[t])
            idxf[s0:s0 + need] = np.maximum(idxf[s0:s0 + need], 0)
    if TRIM == "on":
        # first 2 tiles of each core gather everything: pad idx 0
        for core in range(NCORES):
            for t in (core * TPC, core * TPC + 1):
                a, b = t * STOT * 128, (t + 1) * STOT * 128
                np.maximum(idxf[a:b], 0, out=idxf[a:b])
    else:
        np.maximum(idxf, 0, out=idxf)

    in_maps = []
    for core in range(NCORES):
        t0, t1 = core * TPC, (core + 1) * TPC
        n = STOT * 128
        A = idxT[t0:t1]
        W = A.reshape(TPC, n // 16, 16).transpose(0, 2, 1)
        W = np.tile(W, (1, 8, 1))
        idx16 = np.ascontiguousarray(
            W.transpose(1, 0, 2).reshape(128, TPC * n // 16))
        xi = np.concatenate(
            [xf[:128, t0 * 128:t1 * 128], xf[128:, t0 * 128:t1 * 128]],
            axis=1)
        m = {
            "xg": xT,
            "xI": np.ascontiguousarray(xi),
            "idx16": idx16,
            "coef": coef16,
            "bias2": bias2,
            "cnt": np.ascontiguousarray(
                cnts[t0:t1].reshape(1, TPC * NP)),
            "oh": np.ascontiguousarray(
                ohT[t0:t1][:, SCH].transpose(2, 0, 1, 3).reshape(128, -1)),
        }
        if DCH:
            m["iota16"] = iota16
            m["rloc"] = np.ascontiguousarray(
                rlocT[t0:t1][:, DCH].transpose(2, 0, 1).reshape(128, -1))
            m["val"] = np.ascontiguousarray(
                valT[t0:t1][:, DCH].transpose(2, 0, 1).reshape(128, -1))
        in_maps.append(m)

    res = run_bass_kernel_spmd(nc, in_maps, core_ids=list(range(NCORES)))
    out = np.concatenate([res.results[c]["out"] for c in range(NCORES)],
                         axis=2)
    return np.ascontiguousarray(out[:, :, :NV])


# revision 21
# speedup vs baseline: 1.8033x; 1.0027x over previous
"""MeshConv (gnn_message_passing) Bass kernel for 8 trn2 NeuronCores.

out[b,o,v] = bias[o] + sum_k coeffs[k,:,o]^T feats_k[b,v,:]
  feats_0 = x^T (identity), feats_{1,2,3} = spmm(L/EW/NS, x)

Strategy: shard output vertices across cores (row-partitioned spmm),
fp16 data path.  Edges sorted by (dest tile, col-half, col) into 128-edge
chunks; per tile one dma_gather per <=8-chunk range pulls 512B fp16 rows
of x^T (int16 indices split lo/hi around row 32768).  One-hot matrices
[edge, row_local]*val are precomputed on the host in fp16 and streamed
from HBM (no DVE work).  Per chunk the PE computes
yT[bc_half, row] += g[edge, bc_half]^T @ oh[edge, row] directly in the
transposed layout, so no PE transposes are needed; identity features are
DMA'd straight from x (natural [bc, v] layout).  Per tile pair the
per-operator coeffs hit yT (free dim 256), bias added on the Scalar
engine, output written as [o, rows] slabs per batch.
"""

import sys

sys.path.insert(0, "/opt/trn_rl_repo")

import numpy as np

import concourse.bass as bass
import concourse.bacc as bacc
import concourse.tile as tile
import concourse.mybir as mybir
from concourse.bass_utils import run_bass_kernel_spmd

NV = 40962
B = 4
C = 64
BC = B * C  # 256
NCORES = 8
NTILE = 328          # 128-row tiles, 328*128 = 41984 >= 40962
NVPAD = NTILE * 128
TPC = NTILE // NCORES  # 41 tiles per core
TPCR = TPC * 128
SPLIT = 32768        # int16 index split point
MAXD = 4             # max chunks per gather call (512 descs: 4 fit the 2048 ring)
NQ = 4               # SWDGE queues (ucode max)

OH_MODE = "hybrid"   # stream most one-hots from HBM, build every DVE_EVERY-th on DVE
DVE_EVERY = 0        # 0: stream all one-hots; else build every Nth on DVE
TRIM = "on"          # "on": runtime-trimmed counts; "regfull": registers but
                     # full counts (debug); "off": static num_idxs

_cache = {}

OPK = ("L", "E", "N")


def _pieces(S):
    """Group-aligned gather pieces: (chunk_off, nchunks, is_hi, grp_start).

    Each piece stays inside one (op, half) group so slot padding is always
    trailing within the piece and can be trimmed via num_idxs_reg."""
    C0s = [c0 for c0, _ in S]
    C1s = [c1 for _, c1 in S]
    groups = [(C0s[0], 0), (C0s[1], 0), (C0s[2], 0),
              (C1s[0], 1), (C1s[1], 1), (C1s[2], 1)]
    pieces = []
    base = 0
    for Cg, hi in groups:
        a = 0
        while a < Cg:
            n = min(MAXD, Cg - a)
            pieces.append((base + a, n, hi, base))
            a += n
        base += Cg
    return pieces


def _build(S, oh_mode):
    """Build the per-core Bass program for ((CL0,CL1),(CE0,CE1),(CN0,CN1))."""
    C0s = [c0 for c0, _ in S]
    C1s = [c1 for _, c1 in S]
    CLO, CHI = sum(C0s), sum(C1s)
    STOT = CLO + CHI
    lo_base = [0, C0s[0], C0s[0] + C0s[1]]
    hi_base = [CLO, CLO + C1s[0], CLO + C1s[0] + C1s[1]]
    op_chunks = {
        k: list(range(lo_base[i], lo_base[i] + C0s[i]))
        + list(range(hi_base[i], hi_base[i] + C1s[i]))
        for i, k in enumerate(OPK)
    }
    f32 = mybir.dt.float32
    f16 = mybir.dt.float16

    nc = bacc.Bacc("TRN2", target_bir_lowering=False, debug=False,
                   num_devices=NCORES, num_swdge_queues=NQ,
                   dynamic_dma_scratch_size=32768)

    xg_d = nc.dram_tensor("xg", [NVPAD, BC], f16, kind="ExternalInput")
    xI_d = nc.dram_tensor("xI", [128, 2 * TPCR], f16, kind="ExternalInput")
    idx_d = nc.dram_tensor("idx16", [128, TPC * STOT * 8], mybir.dt.int16,
                           kind="ExternalInput")
    DCH = [cj for cj in range(STOT)
           if DVE_EVERY and cj % DVE_EVERY == 0]
    SCH = [cj for cj in range(STOT) if cj not in DCH]
    ND, NS = len(DCH), len(SCH)
    chpos = {}
    for i, cj in enumerate(DCH):
        chpos[cj] = ("v", i)
    for i, cj in enumerate(SCH):
        chpos[cj] = ("s", i)
    oh_d = nc.dram_tensor("oh", [128, TPC * NS * 128], f16,
                          kind="ExternalInput")
    if ND:
        iota_d = nc.dram_tensor("iota16", [128, 128], f16,
                                kind="ExternalInput")
        rloc_d = nc.dram_tensor("rloc", [128, TPC * ND], f32,
                                kind="ExternalInput")
        val_d = nc.dram_tensor("val", [128, TPC * ND], f32,
                               kind="ExternalInput")
    cnt_d = nc.dram_tensor("cnt", [1, TPC * len(_pieces(S))], mybir.dt.int32,
                           kind="ExternalInput")
    coef_d = nc.dram_tensor("coef", [128, 256], f16, kind="ExternalInput")
    bias_d = nc.dram_tensor("bias2", [64, 1], f32, kind="ExternalInput")
    out_d = nc.dram_tensor("out", [B, C, TPC * 128], f32,
                           kind="ExternalOutput")

    pieces = _pieces(S)
    NP = len(pieces)

    with tile.TileContext(nc) as tc:
        with (
            tc.tile_pool(name="const", bufs=1) as cpool,
            tc.tile_pool(name="meta", bufs=1) as mpool,
            tc.tile_pool(name="g", bufs=3) as gpool,
            tc.tile_pool(name="oh", bufs=3) as ohpool,
            tc.tile_pool(name="yt", bufs=2) as ytpool,
            tc.tile_pool(name="os", bufs=2) as ospool,
            tc.tile_pool(name="py", bufs=1, space="PSUM") as pypool,
            tc.tile_pool(name="po", bufs=2, space="PSUM") as popool,
        ):
            # constants
            coef_t = cpool.tile([128, 256], f16)
            nc.sync.dma_start(coef_t[:], coef_d.ap()[:])
            bias_t = cpool.tile([64, 1], f32)
            nc.sync.dma_start(bias_t[:], bias_d.ap()[:])
            idx_t = mpool.tile([128, TPC * STOT * 8], mybir.dt.int16)
            nc.sync.dma_start(idx_t[:], idx_d.ap()[:])
            cnt_t = mpool.tile([1, TPC * NP], mybir.dt.int32)
            nc.sync.dma_start(cnt_t[:], cnt_d.ap()[:])
            if ND:
                iota_t = cpool.tile([128, 128], f16)
                nc.sync.dma_start(iota_t[:], iota_d.ap()[:])
                rloc_t = mpool.tile([128, TPC * ND], f32)
                nc.sync.dma_start(rloc_t[:], rloc_d.ap()[:])
                val_t = mpool.tile([128, TPC * ND], f32)
                nc.sync.dma_start(val_t[:], val_d.ap()[:])

            yts = {}  # (k, hb) -> staging tile [128, 256] f16 per tile pair
            qn = 0
            creg = nc.gpsimd.alloc_register("cnt_reg")

            for t in range(TPC):
                pair_off = (t % 2) * 128
                is_pair_start = t % 2 == 0
                is_orphan = t == TPC - 1 and is_pair_start

                g_t = gpool.tile([128, STOT * BC], f16, tag="g")
                ib = t * STOT * 8
                for pi, (coff, nch, hi, _) in enumerate(pieces):
                    src = xg_d.ap()[SPLIT:, :] if hi else xg_d.ap()[:SPLIT, :]
                    if t < 2 or TRIM == "off":
                        nreg = nch * 128
                    else:
                        ci = t * NP + pi
                        nc.gpsimd.reg_load(creg, cnt_t[0:1, ci:ci + 1])
                        nreg = creg
                    nc.gpsimd.dma_gather(
                        out_ap=g_t[:, coff * BC:(coff + nch) * BC]
                        .rearrange("p (j f) -> p j f", f=BC),
                        in_ap=src,
                        idxs_ap=idx_t[:, ib + coff * 8:ib + (coff + nch) * 8],
                        num_idxs=nch * 128,
                        num_idxs_reg=nreg,
                        elem_size=BC,
                        queue_num=qn % NQ,
                    )
                    qn += 1

                oh_s = ohpool.tile([128, NS * 128], f16, tag="ohs")
                nc.scalar.dma_start(
                    oh_s[:], oh_d.ap()[:, t * NS * 128:(t + 1) * NS * 128])
                oh_v = (ohpool.tile([128, ND * 128], f16, tag="ohv")
                        if ND else None)
                for i in range(ND):
                    mcol = t * ND + i
                    nc.vector.tensor_scalar(
                        out=oh_v[:, i * 128:(i + 1) * 128],
                        in0=iota_t[:],
                        scalar1=rloc_t[:, mcol:mcol + 1],
                        scalar2=val_t[:, mcol:mcol + 1],
                        op0=mybir.AluOpType.is_equal,
                        op1=mybir.AluOpType.mult,
                    )

                def oh_ap(cj):
                    kind, i = chpos[cj]
                    tl = oh_v if kind == "v" else oh_s
                    return tl[:, i * 128:(i + 1) * 128]

                if is_pair_start:
                    for k in ("I",) + OPK:
                        for hb in (0, 1):
                            yts[(k, hb)] = ytpool.tile(
                                [128, 256], f16, tag=f"yt{k}{hb}",
                                name=f"yt{k}{hb}_{t}")
                            if is_orphan:
                                nc.vector.memset(
                                    yts[(k, hb)][:].bitcast(f32), 0.0)

                # identity features straight from x (natural layout)
                for hb in (0, 1):
                    nc.sync.dma_start(
                        yts[("I", hb)][:, pair_off:pair_off + 128],
                        xI_d.ap()[:, hb * TPCR + t * 128:
                                  hb * TPCR + (t + 1) * 128])

                # spmm: yT[bc_half, row] += g[e, bc_half]^T @ oh[e, row]
                for k in OPK:
                    chunks = op_chunks[k]
                    for hb in (0, 1):
                        py_t = pypool.tile([128, 128], f32, tag=f"py{k}{hb}")
                        for ci, cj in enumerate(chunks):
                            nc.tensor.matmul(
                                py_t[:],
                                g_t[:, cj * BC + hb * 128:
                                    cj * BC + hb * 128 + 128],
                                oh_ap(cj),
                                start=(ci == 0),
                                stop=(ci == len(chunks) - 1),
                            )
                        nc.scalar.activation(
                            yts[(k, hb)][:, pair_off:pair_off + 128], py_t[:],
                            mybir.ActivationFunctionType.Copy)

                # coeffs matmuls on completed pair
                if not is_pair_start or is_orphan:
                    r0 = (t - 1 if not is_pair_start else t) * 128
                    ncols = 128 if is_orphan else 256
                    for b in range(B):
                        po_t = popool.tile([64, 256], f32, tag="po",
                                           name=f"po{b}_{t}")
                        for ki, k in enumerate(("I",) + OPK):
                            p0 = (b % 2) * 64
                            rhs = yts[(k, b // 2)][p0:p0 + 64, :]
                            nc.tensor.matmul(
                                po_t[:],
                                coef_t[p0:p0 + 64, ki * 64:(ki + 1) * 64],
                                rhs,
                                start=(ki == 0),
                                stop=(ki == 3),
                            )
                        os_t = ospool.tile([64, 256], f32, tag="os",
                                           name=f"os{b}_{t}")
                        nc.scalar.activation(
                            os_t[:], po_t[:],
                            mybir.ActivationFunctionType.Identity,
                            bias=bias_t[:, :1])
                        nc.sync.dma_start(
                            out_d.ap()[b:b + 1, :, r0:r0 + ncols]
                            .rearrange("b o r -> (b o) r"),
                            os_t[:, :ncols])

    nc.compile()
    return nc


def _prep(inputs):
    """Sort edges by (tile, col-half, col); compute global chunk counts and
    fill flat slot arrays (idx, one-hot / rloc+val)."""
    ops = []
    for name in ("L", "EW", "NS"):
        row = np.asarray(inputs[f"{name}_row"]).astype(np.int64)
        col = np.asarray(inputs[f"{name}_col"]).astype(np.int64)
        val = np.asarray(inputs[f"{name}_val"]).astype(np.float32)
        t = row >> 7
        h = (col >= SPLIT).astype(np.int64)
        order = np.lexsort((col, h, t))
        row, col, val, t, h = (a[order] for a in (row, col, val, t, h))
        grp = t * 2 + h
        counts = np.bincount(grp, minlength=NTILE * 2)
        starts = np.zeros(NTILE * 2, np.int64)
        starts[1:] = np.cumsum(counts)[:-1]
        pos = np.arange(len(row)) - starts[grp]
        C0 = int(np.ceil(max(int(counts[0::2].max()), 1) / 128))
        C1 = int(np.ceil(max(int(counts[1::2].max()), 1) / 128))
        ops.append((row, col, val, t, h, pos, C0, C1))

    C0s = [o[6] for o in ops]
    C1s = [o[7] for o in ops]
    CLO = sum(C0s)
    STOT = CLO + sum(C1s)
    lo_base = [0, C0s[0], C0s[0] + C0s[1]]
    hi_base = [CLO, CLO + C1s[0], CLO + C1s[0] + C1s[1]]

    idxf = np.full(NTILE * STOT * 128, -1, np.int16)
    ohf = np.zeros((NTILE * STOT * 128, 128), np.float16)
    rlocf = np.zeros(NTILE * STOT * 128, np.float32)
    valf = np.zeros(NTILE * STOT * 128, np.float32)
    gcnt = np.zeros((NTILE, 6), np.int64)  # per (tile, group) edge counts
    for i, (row, col, val, t, h, pos, _, _) in enumerate(ops):
        cb = np.where(h == 0, lo_base[i], hi_base[i])
        slot = (t * STOT + cb) * 128 + pos
        idxf[slot] = (col - h * SPLIT).astype(np.int16)
        ohf[slot, row & 127] = val.astype(np.float16)
        rlocf[slot] = (row & 127).astype(np.float32)
        valf[slot] = val.astype(np.float32)
        grp = i + np.where(h == 0, 0, 3)
        np.add.at(gcnt, (t, grp), 1)

    S = tuple(zip(C0s, C1s))
    return S, STOT, idxf, ohf, rlocf, valf, gcnt


def kernel(**inputs):
    x = np.asarray(inputs["x"], dtype=np.float32)
    coeffs = np.asarray(inputs["coeffs"], dtype=np.float32)
    bias = np.asarray(inputs["bias"], dtype=np.float32)

    S, STOT, idxf, ohf, rlocf, valf, gcnt = _prep(inputs)

    key = (S, OH_MODE)
    if key not in _cache:
        _cache[key] = _build(S, OH_MODE)
    nc = _cache[key]

    xT = np.zeros((NVPAD, BC), np.float16)
    xT[:NV] = x.transpose(2, 0, 1).reshape(NV, BC)
    xf = np.zeros((BC, NVPAD), np.float16)
    xf[:, :NV] = x.reshape(BC, NV)
    coef16 = np.tile(
        coeffs.transpose(1, 0, 2).reshape(64, 256).astype(np.float16), (2, 1))
    bias2 = bias.reshape(64, 1).astype(np.float32)
    iota16 = np.broadcast_to(
        np.arange(128, dtype=np.float16), (128, 128)).copy()

    idxT = idxf.reshape(NTILE, STOT * 128)
    ohT = ohf.reshape(NTILE, STOT, 128, 128)
    rlocT = rlocf.reshape(NTILE, STOT, 128)
    valT = valf.reshape(NTILE, STOT, 128)
    DCH = [cj for cj in range(STOT)
           if DVE_EVERY and cj % DVE_EVERY == 0]
    SCH = [cj for cj in range(STOT) if cj not in DCH]
    pieces = _pieces(S)
    NP = len(pieces)
    # per (tile, piece) valid counts, trailing-trimmed, floored at 16
    cnts = np.zeros((NTILE, NP), np.int32)
    grp_of_piece = []
    C0s = [c0 for c0, _ in S]
    C1s = [c1 for _, c1 in S]
    gbases = np.cumsum([0] + [C0s[0], C0s[1], C0s[2], C1s[0], C1s[1]])
    for pi, (coff, nch, hi, gbase) in enumerate(pieces):
        gi = int(np.searchsorted(gbases, gbase, side="right") - 1)
        start = (coff - gbase) * 128
        c = np.clip(gcnt[:, gi] - start, 0, nch * 128)
        if TRIM == "regfull":
            cnts[:, pi] = nch * 128
            continue
        cnts[:, pi] = np.maximum(c, 16)
        # ensure the >=16 floor has non-negative idxs to match the count
        for t in np.nonzero(c < 16)[0]:
            s0 = (t * STOT + coff) * 128 + int(c[t])
            need = 16 - int(c[t])
            idxf[s0:s0 + need] = np.maximum(idxf[s0:s0 + need], 0)
    if TRIM == "on":
        # first 2 tiles of each core gather everything: pad idx 0
        for core in range(NCORES):
            for t in (core * TPC, core * TPC + 1):
                a, b = t * STOT * 128, (t + 1) * STOT * 128
                np.maximum(idxf[a:b], 0, out=idxf[a:b])
    else:
        np.maximum(idxf, 0, out=idxf)

    in_maps = []
    for core in range(NCORES):
        t0, t1 = core * TPC, (core + 1) * TPC
        n = STOT * 128
        A = idxT[t0:t1]
        W = A.reshape(TPC, n // 16, 16).transpose(0, 2, 1)
        W = np.tile(W, (1, 8, 1))
        idx16 = np.ascontiguousarray(
            W.transpose(1, 0, 2).reshape(128, TPC * n // 16))
        xi = np.concatenate(
            [xf[:128, t0 * 128:t1 * 128], xf[128:, t0 * 128:t1 * 128]],
            axis=1)
        m = {
            "xg": xT,
            "xI": np.ascontiguousarray(xi),
            "idx16": idx16,
            "coef": coef16,
            "bias2": bias2,
            "cnt": np.ascontiguousarray(
                cnts[t0:t1].reshape(1, TPC * NP)),
            "oh": np.ascontiguousarray(
                ohT[t0:t1][:, SCH].transpose(2, 0, 1, 3).reshape(128, -1)),
        }
        if DCH:
            m["iota16"] = iota16
            m["rloc"] = np.ascontiguousarray(
                rlocT[t0:t1][:, DCH].transpose(2, 0, 1).reshape(128, -1))
            m["val"] = np.ascontiguousarray(
                valT[t0:t1][:, DCH].transpose(2, 0, 1).reshape(128, -1))
        in_maps.append(m)

    res = run_bass_kernel_spmd(nc, in_maps, core_ids=list(range(NCORES)))
    out = np.concatenate([res.results[c]["out"] for c in range(NCORES)],
                         axis=2)
    return np.ascontiguousarray(out[:, :, :NV])
